# revision 48
# baseline (speedup 1.0000x reference)
"""Trainium2 Bass kernel for nn_Decoder (MusicVAE-style hierarchical decoder).

Strategy (8 NeuronCores, data-parallel over batch, no inter-core comms):
  - Conductor LSTM (16 sequential levels, batch 32/core) computes per-level
    embeddings; decoder levels are independent, so all 16 levels are batched:
    512 decoder rows per core, 16 sequential note steps.
  - fp8(e4m3) matmuls, DoubleRow perf mode for 512-row streams; fp32 PSUM.
  - Gate chunks are laid out p-adjacent in order (i, f, o, g) so the three
    sigmoid gates batch into one activation op and (i,f) / (o,g) pairs map
    onto two 2-bank PSUM accumulation tiles.
  - The g-gate's conductor-embedding contribution (emb @ dec_Wih_g.T) is
    recomputed on the PE every step (cheaper than a DVE add at model rates);
    its bias rides the tanh activation's bias port.  i/f/o biases are folded
    into ge / gz via Identity-activation copies (no bias matmuls, no ones).
  - Elementwise work is spread across DVE (vector), Pool (gpsimd) and
    Activation engines to balance the per-step makespan against the PE.
"""
import numpy as np
import ml_dtypes

import concourse.bacc as bacc
import concourse.tile as tile
import concourse.mybir as mybir
from concourse.bass_utils import run_bass_kernel_spmd

bf16 = ml_dtypes.bfloat16
f8 = ml_dtypes.float8_e4m3
F32 = mybir.dt.float32
BF = mybir.dt.bfloat16
F8 = mybir.dt.float8e4
AF = mybir.ActivationFunctionType
DR = mybir.MatmulPerfMode.DoubleRow

NCORES = 8
B, Z, H, T = 256, 512, 1024, 512
L, NS = 16, 16
Bc = B // NCORES            # 32 batch rows per core
R = L * Bc                  # 512 decoder rows per core (levels x batch)
HK, TK, ZK = H // 128, T // 128, Z // 128   # 8, 4, 4
G = 4 * H // 128            # 32 gate chunks of 128


def _declare(nc):
    d = {}
    ei = dict(kind="ExternalInput")
    d["cbt"] = nc.dram_tensor("cbt", [128, G], F32, **ei)
    d["dbg"] = nc.dram_tensor("dbg", [128, HK], F32, **ei)
    d["obias"] = nc.dram_tensor("obias", [128, TK], F32, **ei)
    d["zT"] = nc.dram_tensor("zT", [128, ZK, R], F8, **ei)
    d["h0T"] = nc.dram_tensor("h0T", [128, HK, R], F8, **ei)
    d["c0T"] = nc.dram_tensor("c0T", [128, HK, R], F32, **ei)
    d["cwih"] = nc.dram_tensor("cwih", [128, ZK, 4 * H], F8, **ei)
    d["cwhh"] = nc.dram_tensor("cwhh", [128, HK, 4 * H], F8, **ei)
    d["ones"] = nc.dram_tensor("ones", [1, Bc], BF, **ei)
    d["gebb"] = nc.dram_tensor("gebb", [1, 3 * H], BF, **ei)
    d["dwe"] = nc.dram_tensor("dwe", [128, HK, 3 * H], F8, **ei)
    d["dweg"] = nc.dram_tensor("dweg", [128, HK, H], F8, **ei)
    d["dwn"] = nc.dram_tensor("dwn", [128, TK, 4 * H], F8, **ei)
    d["dwhh"] = nc.dram_tensor("dwhh", [128, HK, 4 * H], F8, **ei)
    d["owt"] = nc.dram_tensor("owt", [128, HK, T], F8, **ei)
    d["outbuf"] = nc.dram_tensor("outbuf", [NS, TK, 128, R], BF,
                                 kind="ExternalOutput")
    return d


def _mm_dr(nc, out, w3, x3, ks, ms, start, stop):
    """DoubleRow fp8 matmul over k-subtile pair (ks, ks+1)."""
    return nc.tensor.matmul(out, w3[:, ks:ks + 2, ms], x3[:, ks:ks + 2, :],
                            start=start, stop=stop, perf_mode=DR)


PHASE_MARKS = []


def _mark(nc, name):
    try:
        PHASE_MARKS.append((name, sum(1 for _ in nc.all_instructions())))
    except Exception:
        pass


def _body(nc, tc, d):
    import contextlib
    with contextlib.ExitStack() as ctx:
        Pp = ctx.enter_context(tc.tile_pool(name="persist", bufs=1))

        t_ob = Pp.tile([128, TK], F32, tag="obias")
        t_dbg = Pp.tile([128, HK], F32, tag="dbg")
        t_emb = Pp.tile([128, HK, R], F8, tag="emb")
        t_h = [Pp.tile([128, HK, R], F8, tag=f"hT{i}", name=f"hT{i}")
               for i in (0, 1)]
        t_c = Pp.tile([128, HK, R], F32, tag="c")
        t_note = Pp.tile([128, TK, R], BF, tag="note")
        t_note8 = Pp.tile([128, TK, R], F8, tag="note8")
        # ge persists through the decoder; filled per-level in the conductor
        t_ge = Pp.tile([128, 3 * HK, R], BF, tag="ge")
        # decoder weights needed at dec00 (loaded during the conductor)
        t_dwhh = Pp.tile([128, HK, 4 * H], F8, tag="dwhh")
        t_dweg = Pp.tile([128, HK, H], F8, tag="dweg")

        # ---------------- conductor (+ per-level ge fill) ----------------
        with tc.tile_pool(name="cond", bufs=1) as Pc, \
             tc.tile_pool(name="ctmp", bufs=2) as Pt:
            # DMA order on the shared engine: gz deps first, then cwhh
            # (needed at level 1), then dwe (ge fills), then the rest.
            t_cwhh = Pc.tile([128, HK, 4 * H], F8, tag="cwhh")
            t_ones = Pc.tile([1, Bc], BF, tag="ones")
            t_gebb = Pc.tile([1, 3 * H], BF, tag="gebb")
            # gz chunks indexed [p, gate] with gate order (i, f, o, g)
            t_gz = Pc.tile([128, HK, 4, R], BF, tag="gz")
            t_cc = Pc.tile([128, HK, Bc], F32, tag="cc")

            # gz = z @ cond_Wih.T + cond_b for all levels at once (fp8 DR);
            # bias folded via Identity-activation copy from PSUM.
            _mark(nc, "gz")
            with tc.tile_pool(name="condz", bufs=1) as Pcz, \
                 tc.tile_pool(name="gzps", bufs=2, space="PSUM") as PSz:
                t_cwih = Pcz.tile([128, ZK, 4 * H], F8, tag="cwih")
                nc.sync.dma_start(t_cwih[:], d["cwih"][:])
                t_zT = Pcz.tile([128, ZK, R], F8, tag="zT")
                nc.sync.dma_start(t_zT[:], d["zT"][:])
                t_cbt = Pcz.tile([128, G], F32, tag="cbt")
                nc.sync.dma_start(t_cbt[:], d["cbt"][:])
                nc.sync.dma_start(t_cwhh[:], d["cwhh"][:])
                nc.sync.dma_start(t_ones[:], d["ones"][:])
                nc.sync.dma_start(t_gebb[:], d["gebb"][:])
                for m in range(G):
                    ms = slice(m * 128, (m + 1) * 128)
                    ps = PSz.tile([128, R], F32, tag="gzp", name="gzp")
                    for k in range(0, ZK, 2):
                        _mm_dr(nc, ps[:], t_cwih, t_zT, k, ms,
                               (k == 0), (k == ZK - 2))
                    nc.scalar.activation(t_gz[:, m // 4, m % 4, :], ps[:],
                                         AF.Identity, bias=t_cbt[:, m:m + 1])

            # sequential levels (fp8 non-DR: 32-row streams), elementwise
            # batched level-wide via strided views; each level's ge slice is
            # computed on the PE right after its emb is ready, filling the
            # PE idle while the next level's elementwise chain runs.
            _mark(nc, "conductor")
            with tc.tile_pool(name="dwepool", bufs=1) as Pdwe, \
                 tc.tile_pool(name="cps", bufs=2, space="PSUM") as PSc, \
                 tc.tile_pool(name="geps", bufs=2, space="PSUM") as PSg:
                # dwe lives only through the conductor levels (ge fills)
                t_dwe = Pdwe.tile([128, HK, 3 * H], F8, tag="dwe")
                nc.sync.dma_start(t_dwe[:], d["dwe"][:])
                # decoder weights/state needed at dec00 load during the levels
                nc.sync.dma_start(t_h[0][:], d["h0T"][:])
                nc.sync.dma_start(t_c[:], d["c0T"][:])
                nc.sync.dma_start(t_ob[:], d["obias"][:])
                nc.sync.dma_start(t_dbg[:], d["dbg"][:])
                nc.sync.dma_start(t_dwhh[:], d["dwhh"][:])
                nc.sync.dma_start(t_dweg[:], d["dweg"][:])

                def ge_fill(lv):
                    cs = slice(lv * Bc, (lv + 1) * Bc)
                    gp = PSg.tile([128, 3 * HK, Bc], F32, tag="gep",
                                  name="gep")
                    for m in range(3 * HK):
                        nc.tensor.matmul(gp[:, m, :],
                                         t_gebb[0:1, m * 128:(m + 1) * 128],
                                         t_ones[:], start=True, stop=False)
                        for k in range(0, HK, 2):
                            nc.tensor.matmul(
                                gp[:, m, :],
                                t_dwe[:, k:k + 2, m * 128:(m + 1) * 128],
                                t_emb[:, k:k + 2, cs],
                                start=False, stop=(k == HK - 2), perf_mode=DR)
                    nc.vector.tensor_copy(t_ge[:, :, cs], gp[:])

                for _crep in range(COND_REPS):
                  for lv in range(L):
                      cs = slice(lv * Bc, (lv + 1) * Bc)
                      ps_prev = slice((lv - 1) * Bc, lv * Bc)
                      tsig = Pt.tile([128, HK, 3, Bc], BF, tag="tsig",
                                     name="tsig")
                      tg = Pt.tile([128, HK, Bc], BF, tag="tg", name="tg")
                      tcn = Pt.tile([128, HK, Bc], BF, tag="tcn", name="tcn")
                      if lv == 0:
                          # h0 == 0: gates are just gz; c0 == 0
                          nc.scalar.activation(tsig[:], t_gz[:, :, 0:3, cs],
                                               AF.Sigmoid)
                          nc.scalar.activation(tg[:], t_gz[:, :, 3, cs],
                                               AF.Tanh)
                          nc.vector.tensor_mul(t_cc[:], tsig[:, :, 0, :],
                                               tg[:])
                          nc.scalar.activation(tcn[:], t_cc[:], AF.Tanh)
                          nc.vector.tensor_mul(t_emb[:, :, cs],
                                               tsig[:, :, 2, :], tcn[:])
                      else:
                          ps = PSc.tile([128, HK, 4, Bc], F32, tag="cgp",
                                        name="cgp")
                          for p in range(HK):
                              for g in range(4):
                                  ms = slice((p * 4 + g) * 128,
                                             (p * 4 + g + 1) * 128)
                                  for k in range(0, HK, 2):
                                      nc.tensor.matmul(
                                          ps[:, p, g, :],
                                          t_cwhh[:, k:k + 2, ms],
                                          t_emb[:, k:k + 2, ps_prev],
                                          start=(k == 0),
                                          stop=(k == HK - 2), perf_mode=DR)
                          # previous level's ge fills the PE while this
                          # level's elementwise chain runs
                          ge_fill(lv - 1)
                          gs = Pt.tile([128, HK, 4, Bc], BF, tag="gs",
                                       name="gs")
                          tm1 = Pt.tile([128, HK, Bc], BF, tag="tm1",
                                        name="tm1")
                          tm2 = Pt.tile([128, HK, Bc], F32, tag="tm2",
                                        name="tm2")
                          # elementwise in two p-halves, pipelined across
                          # engines; emb half 0 lands early so the next
                          # level's k-outer matmuls can begin
                          for hp in (slice(0, HK // 2), slice(HK // 2, HK)):
                              nc.vector.tensor_add(gs[:, hp, :, :],
                                                   ps[:, hp, :, :],
                                                   t_gz[:, hp, :, cs])
                              nc.scalar.activation(tsig[:, hp, :, :],
                                                   gs[:, hp, 0:3, :],
                                                   AF.Sigmoid)
                              nc.scalar.activation(tg[:, hp, :],
                                                   gs[:, hp, 3, :], AF.Tanh)
                              nc.vector.tensor_mul(tm1[:, hp, :],
                                                   tsig[:, hp, 0, :],
                                                   tg[:, hp, :])
                              nc.gpsimd.tensor_mul(tm2[:, hp, :],
                                                   tsig[:, hp, 1, :],
                                                   t_cc[:, hp, :])
                              nc.vector.tensor_add(t_cc[:, hp, :],
                                                   tm1[:, hp, :],
                                                   tm2[:, hp, :])
                              nc.scalar.activation(tcn[:, hp, :],
                                                   t_cc[:, hp, :], AF.Tanh)
                              nc.vector.tensor_mul(t_emb[:, hp, cs],
                                                   tsig[:, hp, 2, :],
                                                   tcn[:, hp, :])
                  ge_fill(L - 1)

        # remaining decoder weights (first used at dec00 outproj / dec01)
        Pw2 = ctx.enter_context(tc.tile_pool(name="wdec2", bufs=1))
        t_dwn = Pw2.tile([128, TK, 4 * H], F8, tag="dwn")
        nc.sync.dma_start(t_dwn[:], d["dwn"][:])
        t_owt = Pw2.tile([128, HK, T], F8, tag="owt")
        nc.sync.dma_start(t_owt[:], d["owt"][:])

        # ---------------- decoder: 16 note steps over 512 rows --------------
        with tc.tile_pool(name="dtmp", bufs=4) as Pdt, \
             tc.tile_pool(name="dps", bufs=3, space="PSUM") as PSd, \
             tc.tile_pool(name="dpso", bufs=2, space="PSUM") as PSo:
            prefetched = {}
            for _drep in range(DEC_REPS):
              for t in range(NS):
                  _mark(nc, f"dec{t:02d}")
                  hin = t_h[t % 2]
                  hout = t_h[(t + 1) % 2]
                  psAB = {}
                  # software-pipelined emission: stage ops of chunk p are
                  # emitted after stage ops of chunk p+1's predecessors so
                  # each engine's FIFO never head-of-line blocks on a
                  # dependency that a later-emitted independent op could fill.
                  tiles = {}

                  def mms(p):
                      psA = PSd.tile([128, 2, R], F32, tag="dgp", name="psA")
                      if p in prefetched:
                          psB = prefetched.pop(p)
                      else:
                          psB = PSd.tile([128, 2, R], F32, tag="dgp",
                                         name="psB")
                          # g gate: emb contribution recomputed on PE
                          for k in range(0, HK, 2):
                              _mm_dr(nc, psB[:, 1, :], t_dweg, t_emb,
                                     k, slice(p * 128, (p + 1) * 128),
                                     (k == 0), False)
                      psAB[p] = (psA, psB)
                      for gi in range(4):
                          pst = psA if gi < 2 else psB
                          sl = gi % 2
                          ms = slice((p * 4 + gi) * 128,
                                     (p * 4 + gi + 1) * 128)
                          for k in range(0, HK, 2):
                              _mm_dr(nc, pst[:, sl, :], t_dwhh, hin, k, ms,
                                     (k == 0 and gi != 3),
                                     (t == 0 and k == HK - 2))
                          if t > 0:
                              for k in range(0, TK, 2):
                                  _mm_dr(nc, pst[:, sl, :], t_dwn, t_note8,
                                         k, ms, False, (k == TK - 2))

                  FULL = slice(0, R)
                  HALVES = (slice(0, R // 2), slice(R // 2, R))

                  def adds(p, cl=FULL, alloc=True):
                      psA, psB = psAB[p]
                      if alloc:
                          gs3 = Pdt.tile([128, 3, R], BF, tag="gs3",
                                         name="gs3")
                          tg = Pdt.tile([128, R], BF, tag="tg", name="tg")
                          tiles[p] = (gs3, tg)
                      gs3, tg = tiles[p]
                      nc.vector.tensor_add(gs3[:, 0:2, cl], psA[:, :, cl],
                                           t_ge[:, 3 * p:3 * p + 2, cl])
                      nc.vector.tensor_add(gs3[:, 2, cl], psB[:, 0, cl],
                                           t_ge[:, 3 * p + 2, cl])
                      nc.scalar.activation(tg[:, cl], psB[:, 1, cl], AF.Tanh,
                                           bias=t_dbg[:, p:p + 1])

                  def acts(p, cl=FULL, alloc=True):
                      if alloc:
                          gs3, tg = tiles[p]
                          tsig = Pdt.tile([128, 3, R], BF, tag="tsig3",
                                          name="tsig3")
                          tiles[p] = (tsig, tg, gs3)
                      tsig, tg, gs3 = tiles[p]
                      nc.scalar.activation(tsig[:, :, cl], gs3[:, :, cl],
                                           AF.Sigmoid)

                  def tail(p, cl=FULL, alloc=True):
                      tsig, tg = tiles[p][0], tiles[p][1]
                      if alloc:
                          tcn = Pdt.tile([128, R], BF, tag="tcn", name="tcn")
                          tm1 = Pdt.tile([128, R], BF, tag="tm1", name="tm1")
                          tm2 = Pdt.tile([128, R], F32, tag="tm2", name="tm2")
                          tiles[(p, 'x')] = (tcn, tm1, tm2)
                      tcn, tm1, tm2 = tiles[(p, 'x')]
                      nc.vector.tensor_mul(tm1[:, cl], tsig[:, 0, cl],
                                           tg[:, cl])
                      nc.gpsimd.tensor_mul(tm2[:, cl], tsig[:, 1, cl],
                                           t_c[:, p, cl])
                      nc.gpsimd.tensor_add(t_c[:, p, cl], tm1[:, cl],
                                           tm2[:, cl])
                      nc.scalar.activation(tcn[:, cl], t_c[:, p, cl], AF.Tanh)
                      nc.vector.tensor_mul(hout[:, p, cl], tsig[:, 2, cl],
                                           tcn[:, cl])

                  for p in range(HK + 3):
                      if p < HK:
                          mms(p)
                          if p < HK - 2:
                              adds(p)
                          else:
                              # last two chunks: half-R ops so the
                              # step-boundary chain pipelines at finer grain
                              adds(p, HALVES[0])
                              adds(p, HALVES[1], alloc=False)
                      if 1 <= p and p - 1 < HK:
                          if p - 1 < HK - 2:
                              acts(p - 1)
                          else:
                              acts(p - 1, HALVES[0])
                              acts(p - 1, HALVES[1], alloc=False)
                      if p >= 3:
                          q = p - 3
                          if q < HK - 2:
                              tail(q)
                          else:
                              tail(q, HALVES[0])
                              tail(q, HALVES[1], alloc=False)
                  # output projection + sigmoid -> note (fp8 mirror first so
                  # the next step's Wn matmuls unblock as early as possible)
                  for tk in range(TK):
                      ts_ = slice(tk * 128, (tk + 1) * 128)
                      po = PSo.tile([128, R], F32, tag="dpo", name="dpo")
                      for k in range(0, HK, 2):
                          _mm_dr(nc, po[:], t_owt, hout, k, ts_,
                                 (k == 0), (k == HK - 2))
                      nc.scalar.activation(t_note8[:, tk, :], po[:],
                                           AF.Sigmoid, bias=t_ob[:, tk:tk + 1])
                      nc.scalar.activation(t_note[:, tk, :], po[:],
                                           AF.Sigmoid, bias=t_ob[:, tk:tk + 1])
                      nc.sync.dma_start(d["outbuf"][t, tk], t_note[:, tk, :])


import os
DEC_REPS = int(os.environ.get("KBENCH_DEC_REPS", "1"))
COND_REPS = int(os.environ.get("KBENCH_COND_REPS", "1"))

_CACHE = {}


def _build():
    if "nc" not in _CACHE:
        nc = bacc.Bacc("TRN2", target_bir_lowering=False, debug=False,
                       num_devices=NCORES)
        d = _declare(nc)
        with tile.TileContext(nc) as tc:
            _body(nc, tc, d)
        nc.compile()
        _CACHE["nc"] = nc
    return _CACHE["nc"]


def _feat_major(W, dt):
    """[J, K] -> [128, K/128, J] (stationary lhsT chunk layout)."""
    J, K = W.shape
    return np.ascontiguousarray(
        W.reshape(J, K // 128, 128).transpose(2, 1, 0)).astype(dt)


def _reorder4(W):
    """[4H, K] rows in PyTorch gate blocks (i,f,g,o) -> p-adjacent chunks in
    order (i,f,o,g): new chunk m = p*4 + {0:i,1:f,2:o,3:g}."""
    K = W.shape[1]
    W4 = W.reshape(4, HK, 128, K)[[0, 1, 3, 2]]
    return np.ascontiguousarray(W4.transpose(1, 0, 2, 3).reshape(4 * H, K))


def _pack_inputs(inputs):
    z = np.asarray(inputs["z"], np.float32)
    dec_h0 = np.asarray(inputs["dec_h0"], np.float32)
    dec_c0 = np.asarray(inputs["dec_c0"], np.float32)
    cond_b = np.asarray(inputs["cond_bih"] + inputs["cond_bhh"], np.float32)
    dec_b = np.asarray(inputs["dec_bih"] + inputs["dec_bhh"], np.float32)
    out_b = np.asarray(inputs["out_b"], np.float32)

    cb4 = cond_b.reshape(4, HK, 128)[[0, 1, 3, 2]]
    cbt = np.ascontiguousarray(cb4.transpose(1, 0, 2).reshape(G, 128).T)
    db4 = dec_b.reshape(4, HK, 128)
    gebb = np.ascontiguousarray(
        db4[[0, 1, 3]].transpose(1, 0, 2).reshape(1, 3 * H))
    dbg = np.ascontiguousarray(db4[2].T)

    We = np.asarray(inputs["dec_Wih"][:, :H], np.float32)
    We4 = We.reshape(4, HK, 128, H)
    We3 = np.ascontiguousarray(
        We4[[0, 1, 3]].transpose(1, 0, 2, 3).reshape(3 * H, H))
    dwe = _feat_major(We3, f8)                      # [128, HK, 3H]
    dweg = _feat_major(np.ascontiguousarray(We4[2].reshape(H, H)), f8)

    shared = {
        "cbt": cbt.astype(np.float32),
        "dbg": dbg.astype(np.float32),
        "ones": np.ones((1, Bc), dtype=bf16),
        "gebb": gebb.astype(bf16),
        "obias": np.ascontiguousarray(out_b.reshape(TK, 128).T).astype(np.float32),
        "cwih": _feat_major(_reorder4(np.asarray(inputs["cond_Wih"], np.float32)), f8),
        "cwhh": _feat_major(_reorder4(np.asarray(inputs["cond_Whh"], np.float32)), f8),
        "dwn": _feat_major(_reorder4(np.asarray(inputs["dec_Wih"][:, H:], np.float32)), f8),
        "dwhh": _feat_major(_reorder4(np.asarray(inputs["dec_Whh"], np.float32)), f8),
        "owt": _feat_major(np.asarray(inputs["out_W"], np.float32), f8),
        "dwe": dwe,
        "dweg": dweg,
    }

    z_lv = z[:, np.arange(L) * L, 0, :]           # [B, L, Z]
    in_maps = []
    for c in range(NCORES):
        bs = slice(c * Bc, (c + 1) * Bc)
        zc = z_lv[bs]                              # [Bc, L, Z]
        zT = np.ascontiguousarray(
            zc.reshape(Bc, L, ZK, 128).transpose(3, 2, 1, 0).reshape(128, ZK, R)
        ).astype(f8)
        h0 = dec_h0[:, bs, :]                      # [L, Bc, H]
        h0T = np.ascontiguousarray(
            h0.reshape(L, Bc, HK, 128).transpose(3, 2, 0, 1).reshape(128, HK, R))
        c0 = dec_c0[:, bs, :]
        c0T = np.ascontiguousarray(
            c0.reshape(L, Bc, HK, 128).transpose(3, 2, 0, 1).reshape(128, HK, R))
        m = dict(shared)
        m["zT"] = zT
        m["h0T"] = h0T.astype(f8)
        m["c0T"] = c0T.astype(np.float32)
        in_maps.append(m)
    return in_maps


def _unpack_outputs(core_outs):
    notes = np.empty((B, L * NS, T), np.float32)
    for c, arr in enumerate(core_outs):
        # arr [NS, TK, 128, R] -> [Bc, L, NS, T]
        a = arr.astype(np.float32).reshape(NS, TK, 128, L, Bc).transpose(4, 3, 0, 1, 2)
        notes[c * Bc:(c + 1) * Bc] = a.reshape(Bc, L, NS, T).reshape(
            Bc, L * NS, T)
    return notes


def kernel(**inputs):
    nc = _build()
    in_maps = _pack_inputs(inputs)
    res = run_bass_kernel_spmd(nc, in_maps, list(range(NCORES)))
    return _unpack_outputs([r["outbuf"] for r in res.results])


# revision 52
# speedup vs baseline: 1.0235x; 1.0235x over previous
"""Trainium2 Bass kernel for nn_Decoder (MusicVAE-style hierarchical decoder).

Strategy (8 NeuronCores, data-parallel over batch, no inter-core comms):
  - Conductor LSTM (16 sequential levels, batch 32/core) computes per-level
    embeddings; decoder levels are independent, so all 16 levels are batched:
    512 decoder rows per core, 16 sequential note steps.
  - fp8(e4m3) matmuls, DoubleRow perf mode for 512-row streams; fp32 PSUM.
  - Gate chunks are laid out p-adjacent in order (i, f, o, g) so the three
    sigmoid gates batch into one activation op and (i,f) / (o,g) pairs map
    onto two 2-bank PSUM accumulation tiles.
  - The g-gate's conductor-embedding contribution (emb @ dec_Wih_g.T) is
    recomputed on the PE every step (cheaper than a DVE add at model rates);
    its bias rides the tanh activation's bias port.  i/f/o biases are folded
    into ge / gz via Identity-activation copies (no bias matmuls, no ones).
  - Elementwise work is spread across DVE (vector), Pool (gpsimd) and
    Activation engines to balance the per-step makespan against the PE.
"""
import numpy as np
import ml_dtypes

import concourse.bacc as bacc
import concourse.tile as tile
import concourse.mybir as mybir
from concourse.bass_utils import run_bass_kernel_spmd

bf16 = ml_dtypes.bfloat16
f8 = ml_dtypes.float8_e4m3
F32 = mybir.dt.float32
BF = mybir.dt.bfloat16
F8 = mybir.dt.float8e4
AF = mybir.ActivationFunctionType
DR = mybir.MatmulPerfMode.DoubleRow

NCORES = 8
B, Z, H, T = 256, 512, 1024, 512
L, NS = 16, 16
Bc = B // NCORES            # 32 batch rows per core
R = L * Bc                  # 512 decoder rows per core (levels x batch)
HK, TK, ZK = H // 128, T // 128, Z // 128   # 8, 4, 4
G = 4 * H // 128            # 32 gate chunks of 128


def _declare(nc):
    d = {}
    ei = dict(kind="ExternalInput")
    d["cbt"] = nc.dram_tensor("cbt", [128, G], F32, **ei)
    d["dbg"] = nc.dram_tensor("dbg", [128, HK], F32, **ei)
    d["obias"] = nc.dram_tensor("obias", [128, TK], F32, **ei)
    d["zT"] = nc.dram_tensor("zT", [128, ZK, R], F8, **ei)
    d["h0T"] = nc.dram_tensor("h0T", [128, HK, R], F8, **ei)
    d["c0T"] = nc.dram_tensor("c0T", [128, HK, R], F32, **ei)
    d["cwih"] = nc.dram_tensor("cwih", [128, ZK, 4 * H], F8, **ei)
    d["cwhh"] = nc.dram_tensor("cwhh", [128, HK, 4 * H], F8, **ei)
    d["ones"] = nc.dram_tensor("ones", [1, Bc], BF, **ei)
    d["gebb"] = nc.dram_tensor("gebb", [1, 3 * H], BF, **ei)
    d["dwe"] = nc.dram_tensor("dwe", [128, HK, 3 * H], F8, **ei)
    d["dweg"] = nc.dram_tensor("dweg", [128, HK, H], F8, **ei)
    d["dwn"] = nc.dram_tensor("dwn", [128, TK, 4 * H], F8, **ei)
    d["dwhh"] = nc.dram_tensor("dwhh", [128, HK, 4 * H], F8, **ei)
    d["owt"] = nc.dram_tensor("owt", [128, HK, T], F8, **ei)
    d["outbuf"] = nc.dram_tensor("outbuf", [NS, TK, 128, R], BF,
                                 kind="ExternalOutput")
    return d


def _mm_dr(nc, out, w3, x3, ks, ms, start, stop):
    """DoubleRow fp8 matmul over k-subtile pair (ks, ks+1)."""
    return nc.tensor.matmul(out, w3[:, ks:ks + 2, ms], x3[:, ks:ks + 2, :],
                            start=start, stop=stop, perf_mode=DR)


PHASE_MARKS = []


def _mark(nc, name):
    try:
        PHASE_MARKS.append((name, sum(1 for _ in nc.all_instructions())))
    except Exception:
        pass


def _body(nc, tc, d):
    import contextlib
    with contextlib.ExitStack() as ctx:
        Pp = ctx.enter_context(tc.tile_pool(name="persist", bufs=1))

        t_ob = Pp.tile([128, TK], F32, tag="obias")
        t_dbg = Pp.tile([128, HK], F32, tag="dbg")
        t_emb = Pp.tile([128, HK, R], F8, tag="emb")
        t_h = [Pp.tile([128, HK, R], F8, tag=f"hT{i}", name=f"hT{i}")
               for i in (0, 1)]
        t_c = Pp.tile([128, HK, R], F32, tag="c")
        t_note = Pp.tile([128, TK, R], BF, tag="note")
        t_note8 = Pp.tile([128, TK, R], F8, tag="note8")
        # ge persists through the decoder; filled per-level in the conductor
        t_ge = Pp.tile([128, 3 * HK, R], BF, tag="ge")
        # decoder weights needed at dec00 (loaded during the conductor)
        t_dwhh = Pp.tile([128, HK, 4 * H], F8, tag="dwhh")
        t_dweg = Pp.tile([128, HK, H], F8, tag="dweg")

        # ---------------- conductor (+ per-level ge fill) ----------------
        with tc.tile_pool(name="cond", bufs=1) as Pc, \
             tc.tile_pool(name="ctmp", bufs=2) as Pt:
            # DMA order on the shared engine: gz deps first, then cwhh
            # (needed at level 1), then dwe (ge fills), then the rest.
            t_cwhh = Pc.tile([128, HK, 4 * H], F8, tag="cwhh")
            t_ones = Pc.tile([1, Bc], BF, tag="ones")
            t_gebb = Pc.tile([1, 3 * H], BF, tag="gebb")
            # gz chunks indexed [p, gate] with gate order (i, f, o, g)
            t_gz = Pc.tile([128, HK, 4, R], BF, tag="gz")
            t_cc = Pc.tile([128, HK, Bc], F32, tag="cc")

            # gz = z @ cond_Wih.T + cond_b for all levels at once (fp8 DR);
            # bias folded via Identity-activation copy from PSUM.
            _mark(nc, "gz")
            with tc.tile_pool(name="condz", bufs=1) as Pcz, \
                 tc.tile_pool(name="gzps", bufs=2, space="PSUM") as PSz:
                t_cwih = Pcz.tile([128, ZK, 4 * H], F8, tag="cwih")
                t_zT = Pcz.tile([128, ZK, R], F8, tag="zT")
                nc.sync.dma_start(t_zT[:], d["zT"][:])
                t_cbt = Pcz.tile([128, G], F32, tag="cbt")
                nc.sync.dma_start(t_cbt[:], d["cbt"][:])
                # cwih in halves so the first gz chunks start ~4us earlier
                half = 2 * H
                nc.sync.dma_start(t_cwih[:, :, 0:half],
                                  d["cwih"][:, :, 0:half])
                nc.sync.dma_start(t_cwih[:, :, half:4 * H],
                                  d["cwih"][:, :, half:4 * H])
                nc.sync.dma_start(t_cwhh[:], d["cwhh"][:])
                nc.sync.dma_start(t_ones[:], d["ones"][:])
                nc.sync.dma_start(t_gebb[:], d["gebb"][:])
                for m in range(G):
                    ms = slice(m * 128, (m + 1) * 128)
                    ps = PSz.tile([128, R], F32, tag="gzp", name="gzp")
                    for k in range(0, ZK, 2):
                        _mm_dr(nc, ps[:], t_cwih, t_zT, k, ms,
                               (k == 0), (k == ZK - 2))
                    nc.scalar.activation(t_gz[:, m // 4, m % 4, :], ps[:],
                                         AF.Identity, bias=t_cbt[:, m:m + 1])

            # sequential levels (fp8 non-DR: 32-row streams), elementwise
            # batched level-wide via strided views; each level's ge slice is
            # computed on the PE right after its emb is ready, filling the
            # PE idle while the next level's elementwise chain runs.
            _mark(nc, "conductor")
            with tc.tile_pool(name="dwepool", bufs=1) as Pdwe, \
                 tc.tile_pool(name="cps", bufs=2, space="PSUM") as PSc, \
                 tc.tile_pool(name="geps", bufs=2, space="PSUM") as PSg:
                # dwe lives only through the conductor levels (ge fills)
                t_dwe = Pdwe.tile([128, HK, 3 * H], F8, tag="dwe")
                nc.sync.dma_start(t_dwe[:], d["dwe"][:])
                # decoder weights/state needed at dec00 load during the levels
                nc.sync.dma_start(t_h[0][:], d["h0T"][:])
                nc.sync.dma_start(t_c[:], d["c0T"][:])
                nc.sync.dma_start(t_ob[:], d["obias"][:])
                nc.sync.dma_start(t_dbg[:], d["dbg"][:])
                nc.sync.dma_start(t_dwhh[:], d["dwhh"][:])
                nc.sync.dma_start(t_dweg[:], d["dweg"][:])

                def ge_fill(lv):
                    cs = slice(lv * Bc, (lv + 1) * Bc)
                    gp = PSg.tile([128, 3 * HK, Bc], F32, tag="gep",
                                  name="gep")
                    for m in range(3 * HK):
                        nc.tensor.matmul(gp[:, m, :],
                                         t_gebb[0:1, m * 128:(m + 1) * 128],
                                         t_ones[:], start=True, stop=False)
                        for k in range(0, HK, 2):
                            nc.tensor.matmul(
                                gp[:, m, :],
                                t_dwe[:, k:k + 2, m * 128:(m + 1) * 128],
                                t_emb[:, k:k + 2, cs],
                                start=False, stop=(k == HK - 2), perf_mode=DR)
                    nc.vector.tensor_copy(t_ge[:, :, cs], gp[:])

                for _crep in range(COND_REPS):
                  for lv in range(L):
                      cs = slice(lv * Bc, (lv + 1) * Bc)
                      ps_prev = slice((lv - 1) * Bc, lv * Bc)
                      tsig = Pt.tile([128, HK, 3, Bc], BF, tag="tsig",
                                     name="tsig")
                      tg = Pt.tile([128, HK, Bc], BF, tag="tg", name="tg")
                      tcn = Pt.tile([128, HK, Bc], BF, tag="tcn", name="tcn")
                      if lv == 0:
                          # h0 == 0: gates are just gz; c0 == 0
                          nc.scalar.activation(tsig[:], t_gz[:, :, 0:3, cs],
                                               AF.Sigmoid)
                          nc.scalar.activation(tg[:], t_gz[:, :, 3, cs],
                                               AF.Tanh)
                          nc.vector.tensor_mul(t_cc[:], tsig[:, :, 0, :],
                                               tg[:])
                          nc.scalar.activation(tcn[:], t_cc[:], AF.Tanh)
                          nc.vector.tensor_mul(t_emb[:, :, cs],
                                               tsig[:, :, 2, :], tcn[:])
                      else:
                          ps = PSc.tile([128, HK, 4, Bc], F32, tag="cgp",
                                        name="cgp")
                          for p in range(HK):
                              for g in range(4):
                                  ms = slice((p * 4 + g) * 128,
                                             (p * 4 + g + 1) * 128)
                                  for k in range(0, HK, 2):
                                      nc.tensor.matmul(
                                          ps[:, p, g, :],
                                          t_cwhh[:, k:k + 2, ms],
                                          t_emb[:, k:k + 2, ps_prev],
                                          start=(k == 0),
                                          stop=(k == HK - 2), perf_mode=DR)
                          # previous level's ge fills the PE while this
                          # level's elementwise chain runs
                          ge_fill(lv - 1)
                          gs = Pt.tile([128, HK, 4, Bc], BF, tag="gs",
                                       name="gs")
                          tm1 = Pt.tile([128, HK, Bc], BF, tag="tm1",
                                        name="tm1")
                          tm2 = Pt.tile([128, HK, Bc], F32, tag="tm2",
                                        name="tm2")
                          # elementwise in two p-halves, pipelined across
                          # engines; emb half 0 lands early so the next
                          # level's k-outer matmuls can begin
                          for hp in (slice(0, HK // 2), slice(HK // 2, HK)):
                              nc.vector.tensor_add(gs[:, hp, :, :],
                                                   ps[:, hp, :, :],
                                                   t_gz[:, hp, :, cs])
                              nc.scalar.activation(tsig[:, hp, :, :],
                                                   gs[:, hp, 0:3, :],
                                                   AF.Sigmoid)
                              nc.scalar.activation(tg[:, hp, :],
                                                   gs[:, hp, 3, :], AF.Tanh)
                              nc.vector.tensor_mul(tm1[:, hp, :],
                                                   tsig[:, hp, 0, :],
                                                   tg[:, hp, :])
                              nc.gpsimd.tensor_mul(tm2[:, hp, :],
                                                   tsig[:, hp, 1, :],
                                                   t_cc[:, hp, :])
                              nc.vector.tensor_add(t_cc[:, hp, :],
                                                   tm1[:, hp, :],
                                                   tm2[:, hp, :])
                              nc.scalar.activation(tcn[:, hp, :],
                                                   t_cc[:, hp, :], AF.Tanh)
                              nc.vector.tensor_mul(t_emb[:, hp, cs],
                                                   tsig[:, hp, 2, :],
                                                   tcn[:, hp, :])
                  ge_fill(L - 1)

        # remaining decoder weights (first used at dec00 outproj / dec01)
        Pw2 = ctx.enter_context(tc.tile_pool(name="wdec2", bufs=1))
        t_dwn = Pw2.tile([128, TK, 4 * H], F8, tag="dwn")
        nc.sync.dma_start(t_dwn[:], d["dwn"][:])
        t_owt = Pw2.tile([128, HK, T], F8, tag="owt")
        nc.sync.dma_start(t_owt[:], d["owt"][:])

        # ---------------- decoder: 16 note steps over 512 rows --------------
        with tc.tile_pool(name="dtmp", bufs=4) as Pdt, \
             tc.tile_pool(name="dps", bufs=3, space="PSUM") as PSd, \
             tc.tile_pool(name="dpso", bufs=2, space="PSUM") as PSo:
            prefetched = {}
            for _drep in range(DEC_REPS):
              for t in range(NS):
                  _mark(nc, f"dec{t:02d}")
                  hin = t_h[t % 2]
                  hout = t_h[(t + 1) % 2]
                  psAB = {}
                  # software-pipelined emission: stage ops of chunk p are
                  # emitted after stage ops of chunk p+1's predecessors so
                  # each engine's FIFO never head-of-line blocks on a
                  # dependency that a later-emitted independent op could fill.
                  tiles = {}

                  def mms(p):
                      psA = PSd.tile([128, 2, R], F32, tag="dgp", name="psA")
                      if p in prefetched:
                          psB = prefetched.pop(p)
                      else:
                          psB = PSd.tile([128, 2, R], F32, tag="dgp",
                                         name="psB")
                          # g gate: emb contribution recomputed on PE
                          for k in range(0, HK, 2):
                              _mm_dr(nc, psB[:, 1, :], t_dweg, t_emb,
                                     k, slice(p * 128, (p + 1) * 128),
                                     (k == 0), False)
                      psAB[p] = (psA, psB)
                      for gi in range(4):
                          pst = psA if gi < 2 else psB
                          sl = gi % 2
                          ms = slice((p * 4 + gi) * 128,
                                     (p * 4 + gi + 1) * 128)
                          for k in range(0, HK, 2):
                              _mm_dr(nc, pst[:, sl, :], t_dwhh, hin, k, ms,
                                     (k == 0 and gi != 3),
                                     (t == 0 and k == HK - 2))
                          if t > 0:
                              for k in range(0, TK, 2):
                                  _mm_dr(nc, pst[:, sl, :], t_dwn, t_note8,
                                         k, ms, False, (k == TK - 2))

                  FULL = slice(0, R)
                  HALVES = (slice(0, R // 2), slice(R // 2, R))

                  def adds(p, cl=FULL, alloc=True):
                      psA, psB = psAB[p]
                      if alloc:
                          gs3 = Pdt.tile([128, 3, R], BF, tag="gs3",
                                         name="gs3")
                          tg = Pdt.tile([128, R], BF, tag="tg", name="tg")
                          tiles[p] = (gs3, tg)
                      gs3, tg = tiles[p]
                      nc.vector.tensor_add(gs3[:, 0:2, cl], psA[:, :, cl],
                                           t_ge[:, 3 * p:3 * p + 2, cl])
                      nc.vector.tensor_add(gs3[:, 2, cl], psB[:, 0, cl],
                                           t_ge[:, 3 * p + 2, cl])
                      nc.scalar.activation(tg[:, cl], psB[:, 1, cl], AF.Tanh,
                                           bias=t_dbg[:, p:p + 1])

                  def acts(p, cl=FULL, alloc=True):
                      if alloc:
                          gs3, tg = tiles[p]
                          tsig = Pdt.tile([128, 3, R], BF, tag="tsig3",
                                          name="tsig3")
                          tiles[p] = (tsig, tg, gs3)
                      tsig, tg, gs3 = tiles[p]
                      nc.scalar.activation(tsig[:, :, cl], gs3[:, :, cl],
                                           AF.Sigmoid)

                  def tail(p, cl=FULL, alloc=True):
                      tsig, tg = tiles[p][0], tiles[p][1]
                      if alloc:
                          tcn = Pdt.tile([128, R], BF, tag="tcn", name="tcn")
                          tm1 = Pdt.tile([128, R], BF, tag="tm1", name="tm1")
                          tm2 = Pdt.tile([128, R], F32, tag="tm2", name="tm2")
                          tiles[(p, 'x')] = (tcn, tm1, tm2)
                      tcn, tm1, tm2 = tiles[(p, 'x')]
                      nc.vector.tensor_mul(tm1[:, cl], tsig[:, 0, cl],
                                           tg[:, cl])
                      nc.gpsimd.tensor_mul(tm2[:, cl], tsig[:, 1, cl],
                                           t_c[:, p, cl])
                      nc.gpsimd.tensor_add(t_c[:, p, cl], tm1[:, cl],
                                           tm2[:, cl])
                      nc.scalar.activation(tcn[:, cl], t_c[:, p, cl], AF.Tanh)
                      nc.vector.tensor_mul(hout[:, p, cl], tsig[:, 2, cl],
                                           tcn[:, cl])

                  for p in range(HK + 3):
                      if p < HK:
                          mms(p)
                          if p < HK - 2:
                              adds(p)
                          else:
                              # last two chunks: half-R ops so the
                              # step-boundary chain pipelines at finer grain
                              adds(p, HALVES[0])
                              adds(p, HALVES[1], alloc=False)
                      if 1 <= p and p - 1 < HK:
                          if p - 1 < HK - 2:
                              acts(p - 1)
                          else:
                              acts(p - 1, HALVES[0])
                              acts(p - 1, HALVES[1], alloc=False)
                      if p >= 3:
                          q = p - 3
                          if q < HK - 2:
                              tail(q)
                          else:
                              tail(q, HALVES[0])
                              tail(q, HALVES[1], alloc=False)
                  # output projection + sigmoid -> note (fp8 mirror first so
                  # the next step's Wn matmuls unblock as early as possible)
                  for tk in range(TK):
                      ts_ = slice(tk * 128, (tk + 1) * 128)
                      po = PSo.tile([128, R], F32, tag="dpo", name="dpo")
                      for k in range(0, HK, 2):
                          _mm_dr(nc, po[:], t_owt, hout, k, ts_,
                                 (k == 0), (k == HK - 2))
                      nc.scalar.activation(t_note[:, tk, :], po[:],
                                           AF.Sigmoid, bias=t_ob[:, tk:tk + 1])
                      if t + 1 < NS:
                          nc.vector.tensor_copy(t_note8[:, tk, :],
                                                t_note[:, tk, :])
                      nc.sync.dma_start(d["outbuf"][t, tk], t_note[:, tk, :])


import os
DEC_REPS = int(os.environ.get("KBENCH_DEC_REPS", "1"))
COND_REPS = int(os.environ.get("KBENCH_COND_REPS", "1"))

_CACHE = {}


def _build():
    if "nc" not in _CACHE:
        nc = bacc.Bacc("TRN2", target_bir_lowering=False, debug=False,
                       num_devices=NCORES)
        d = _declare(nc)
        with tile.TileContext(nc) as tc:
            _body(nc, tc, d)
        nc.compile()
        _CACHE["nc"] = nc
    return _CACHE["nc"]


def _feat_major(W, dt):
    """[J, K] -> [128, K/128, J] (stationary lhsT chunk layout)."""
    J, K = W.shape
    return np.ascontiguousarray(
        W.reshape(J, K // 128, 128).transpose(2, 1, 0)).astype(dt)


def _reorder4(W):
    """[4H, K] rows in PyTorch gate blocks (i,f,g,o) -> p-adjacent chunks in
    order (i,f,o,g): new chunk m = p*4 + {0:i,1:f,2:o,3:g}."""
    K = W.shape[1]
    W4 = W.reshape(4, HK, 128, K)[[0, 1, 3, 2]]
    return np.ascontiguousarray(W4.transpose(1, 0, 2, 3).reshape(4 * H, K))


def _pack_inputs(inputs):
    z = np.asarray(inputs["z"], np.float32)
    dec_h0 = np.asarray(inputs["dec_h0"], np.float32)
    dec_c0 = np.asarray(inputs["dec_c0"], np.float32)
    cond_b = np.asarray(inputs["cond_bih"] + inputs["cond_bhh"], np.float32)
    dec_b = np.asarray(inputs["dec_bih"] + inputs["dec_bhh"], np.float32)
    out_b = np.asarray(inputs["out_b"], np.float32)

    cb4 = cond_b.reshape(4, HK, 128)[[0, 1, 3, 2]]
    cbt = np.ascontiguousarray(cb4.transpose(1, 0, 2).reshape(G, 128).T)
    db4 = dec_b.reshape(4, HK, 128)
    gebb = np.ascontiguousarray(
        db4[[0, 1, 3]].transpose(1, 0, 2).reshape(1, 3 * H))
    dbg = np.ascontiguousarray(db4[2].T)

    We = np.asarray(inputs["dec_Wih"][:, :H], np.float32)
    We4 = We.reshape(4, HK, 128, H)
    We3 = np.ascontiguousarray(
        We4[[0, 1, 3]].transpose(1, 0, 2, 3).reshape(3 * H, H))
    dwe = _feat_major(We3, f8)                      # [128, HK, 3H]
    dweg = _feat_major(np.ascontiguousarray(We4[2].reshape(H, H)), f8)

    shared = {
        "cbt": cbt.astype(np.float32),
        "dbg": dbg.astype(np.float32),
        "ones": np.ones((1, Bc), dtype=bf16),
        "gebb": gebb.astype(bf16),
        "obias": np.ascontiguousarray(out_b.reshape(TK, 128).T).astype(np.float32),
        "cwih": _feat_major(_reorder4(np.asarray(inputs["cond_Wih"], np.float32)), f8),
        "cwhh": _feat_major(_reorder4(np.asarray(inputs["cond_Whh"], np.float32)), f8),
        "dwn": _feat_major(_reorder4(np.asarray(inputs["dec_Wih"][:, H:], np.float32)), f8),
        "dwhh": _feat_major(_reorder4(np.asarray(inputs["dec_Whh"], np.float32)), f8),
        "owt": _feat_major(np.asarray(inputs["out_W"], np.float32), f8),
        "dwe": dwe,
        "dweg": dweg,
    }

    z_lv = z[:, np.arange(L) * L, 0, :]           # [B, L, Z]
    in_maps = []
    for c in range(NCORES):
        bs = slice(c * Bc, (c + 1) * Bc)
        zc = z_lv[bs]                              # [Bc, L, Z]
        zT = np.ascontiguousarray(
            zc.reshape(Bc, L, ZK, 128).transpose(3, 2, 1, 0).reshape(128, ZK, R)
        ).astype(f8)
        h0 = dec_h0[:, bs, :]                      # [L, Bc, H]
        h0T = np.ascontiguousarray(
            h0.reshape(L, Bc, HK, 128).transpose(3, 2, 0, 1).reshape(128, HK, R))
        c0 = dec_c0[:, bs, :]
        c0T = np.ascontiguousarray(
            c0.reshape(L, Bc, HK, 128).transpose(3, 2, 0, 1).reshape(128, HK, R))
        m = dict(shared)
        m["zT"] = zT
        m["h0T"] = h0T.astype(f8)
        m["c0T"] = c0T.astype(np.float32)
        in_maps.append(m)
    return in_maps


def _unpack_outputs(core_outs):
    notes = np.empty((B, L * NS, T), np.float32)
    for c, arr in enumerate(core_outs):
        # arr [NS, TK, 128, R] -> [Bc, L, NS, T]
        a = arr.astype(np.float32).reshape(NS, TK, 128, L, Bc).transpose(4, 3, 0, 1, 2)
        notes[c * Bc:(c + 1) * Bc] = a.reshape(Bc, L, NS, T).reshape(
            Bc, L * NS, T)
    return notes


def kernel(**inputs):
    nc = _build()
    in_maps = _pack_inputs(inputs)
    res = run_bass_kernel_spmd(nc, in_maps, list(range(NCORES)))
    return _unpack_outputs([r["outbuf"] for r in res.results])


# revision 54
# speedup vs baseline: 1.0449x; 1.0210x over previous
"""Trainium2 Bass kernel for nn_Decoder (MusicVAE-style hierarchical decoder).

Strategy (8 NeuronCores, data-parallel over batch, no inter-core comms):
  - Conductor LSTM (16 sequential levels, batch 32/core) computes per-level
    embeddings; decoder levels are independent, so all 16 levels are batched:
    512 decoder rows per core, 16 sequential note steps.
  - fp8(e4m3) matmuls, DoubleRow perf mode for 512-row streams; fp32 PSUM.
  - Gate chunks are laid out p-adjacent in order (i, f, o, g) so the three
    sigmoid gates batch into one activation op and (i,f) / (o,g) pairs map
    onto two 2-bank PSUM accumulation tiles.
  - The g-gate's conductor-embedding contribution (emb @ dec_Wih_g.T) is
    recomputed on the PE every step (cheaper than a DVE add at model rates);
    its bias rides the tanh activation's bias port.  i/f/o biases are folded
    into ge / gz via Identity-activation copies (no bias matmuls, no ones).
  - Elementwise work is spread across DVE (vector), Pool (gpsimd) and
    Activation engines to balance the per-step makespan against the PE.
"""
import numpy as np
import ml_dtypes

import concourse.bacc as bacc
import concourse.tile as tile
import concourse.mybir as mybir
from concourse.bass_utils import run_bass_kernel_spmd

bf16 = ml_dtypes.bfloat16
f8 = ml_dtypes.float8_e4m3
F32 = mybir.dt.float32
BF = mybir.dt.bfloat16
F8 = mybir.dt.float8e4
AF = mybir.ActivationFunctionType
DR = mybir.MatmulPerfMode.DoubleRow

NCORES = 8
B, Z, H, T = 256, 512, 1024, 512
L, NS = 16, 16
Bc = B // NCORES            # 32 batch rows per core
R = L * Bc                  # 512 decoder rows per core (levels x batch)
HK, TK, ZK = H // 128, T // 128, Z // 128   # 8, 4, 4
G = 4 * H // 128            # 32 gate chunks of 128


def _declare(nc):
    d = {}
    ei = dict(kind="ExternalInput")
    d["cbt"] = nc.dram_tensor("cbt", [128, G], F32, **ei)
    d["dbg"] = nc.dram_tensor("dbg", [128, HK], F32, **ei)
    d["obias"] = nc.dram_tensor("obias", [128, TK], F32, **ei)
    d["zT"] = nc.dram_tensor("zT", [128, ZK, R], F8, **ei)
    d["h0T"] = nc.dram_tensor("h0T", [128, HK, R], F8, **ei)
    d["c0T"] = nc.dram_tensor("c0T", [128, HK, R], F32, **ei)
    d["cwih"] = nc.dram_tensor("cwih", [128, ZK, 4 * H], F8, **ei)
    d["cwhh"] = nc.dram_tensor("cwhh", [128, HK, 4 * H], F8, **ei)
    d["ones"] = nc.dram_tensor("ones", [1, Bc], BF, **ei)
    d["gebb"] = nc.dram_tensor("gebb", [1, 3 * H], BF, **ei)
    d["dwe"] = nc.dram_tensor("dwe", [128, HK, 3 * H], F8, **ei)
    d["dweg"] = nc.dram_tensor("dweg", [128, HK, H], F8, **ei)
    d["dwn"] = nc.dram_tensor("dwn", [128, TK, 4 * H], F8, **ei)
    d["dwhh"] = nc.dram_tensor("dwhh", [128, HK, 4 * H], F8, **ei)
    d["owt"] = nc.dram_tensor("owt", [128, HK, T], F8, **ei)
    d["outbuf"] = nc.dram_tensor("outbuf", [NS, TK, 128, R], BF,
                                 kind="ExternalOutput")
    return d


def _mm_dr(nc, out, w3, x3, ks, ms, start, stop):
    """DoubleRow fp8 matmul over k-subtile pair (ks, ks+1)."""
    return nc.tensor.matmul(out, w3[:, ks:ks + 2, ms], x3[:, ks:ks + 2, :],
                            start=start, stop=stop, perf_mode=DR)


PHASE_MARKS = []


def _mark(nc, name):
    try:
        PHASE_MARKS.append((name, sum(1 for _ in nc.all_instructions())))
    except Exception:
        pass


def _body(nc, tc, d):
    import contextlib
    with contextlib.ExitStack() as ctx:
        Pp = ctx.enter_context(tc.tile_pool(name="persist", bufs=1))

        t_ob = Pp.tile([128, TK], F32, tag="obias")
        t_dbg = Pp.tile([128, HK], F32, tag="dbg")
        t_emb = Pp.tile([128, HK, R], F8, tag="emb")
        t_h = [Pp.tile([128, HK, R], F8, tag=f"hT{i}", name=f"hT{i}")
               for i in (0, 1)]
        t_c = Pp.tile([128, HK, R], F32, tag="c")
        t_note = Pp.tile([128, TK, R], BF, tag="note")
        t_note8 = Pp.tile([128, TK, R], F8, tag="note8")
        # ge persists through the decoder; filled per-level in the conductor
        t_ge = Pp.tile([128, 3 * HK, R], BF, tag="ge")
        # decoder weights needed at dec00 (loaded during the conductor)
        t_dwhh = Pp.tile([128, HK, 4 * H], F8, tag="dwhh")
        t_dweg = Pp.tile([128, HK, H], F8, tag="dweg")

        # ---------------- conductor (+ per-level ge fill) ----------------
        with tc.tile_pool(name="cond", bufs=1) as Pc, \
             tc.tile_pool(name="ctmp", bufs=2) as Pt:
            # DMA order on the shared engine: gz deps first, then cwhh
            # (needed at level 1), then dwe (ge fills), then the rest.
            t_cwhh = Pc.tile([128, HK, 4 * H], F8, tag="cwhh")
            t_ones = Pc.tile([1, Bc], BF, tag="ones")
            t_gebb = Pc.tile([1, 3 * H], BF, tag="gebb")
            # gz chunks indexed [p, gate] with gate order (i, f, o, g)
            t_gz = Pc.tile([128, HK, 4, R], BF, tag="gz")
            t_cc = Pc.tile([128, HK, Bc], F32, tag="cc")

            # gz = z @ cond_Wih.T + cond_b for all levels at once (fp8 DR);
            # bias folded via Identity-activation copy from PSUM.
            _mark(nc, "gz")
            with tc.tile_pool(name="condz", bufs=1) as Pcz, \
                 tc.tile_pool(name="gzps", bufs=2, space="PSUM") as PSz:
                t_cwih = Pcz.tile([128, ZK, 4 * H], F8, tag="cwih")
                t_zT = Pcz.tile([128, ZK, R], F8, tag="zT")
                nc.sync.dma_start(t_zT[:], d["zT"][:])
                t_cbt = Pcz.tile([128, G], F32, tag="cbt")
                nc.sync.dma_start(t_cbt[:], d["cbt"][:])
                # cwih in halves so the first gz chunks start ~4us earlier
                half = 2 * H
                nc.sync.dma_start(t_cwih[:, :, 0:half],
                                  d["cwih"][:, :, 0:half])
                nc.sync.dma_start(t_cwih[:, :, half:4 * H],
                                  d["cwih"][:, :, half:4 * H])
                nc.sync.dma_start(t_cwhh[:], d["cwhh"][:])
                nc.sync.dma_start(t_ones[:], d["ones"][:])
                nc.sync.dma_start(t_gebb[:], d["gebb"][:])
                for m in range(G):
                    ms = slice(m * 128, (m + 1) * 128)
                    ps = PSz.tile([128, R], F32, tag="gzp", name="gzp")
                    for k in range(0, ZK, 2):
                        _mm_dr(nc, ps[:], t_cwih, t_zT, k, ms,
                               (k == 0), (k == ZK - 2))
                    nc.scalar.activation(t_gz[:, m // 4, m % 4, :], ps[:],
                                         AF.Identity, bias=t_cbt[:, m:m + 1])

            # sequential levels (fp8 non-DR: 32-row streams), elementwise
            # batched level-wide via strided views; each level's ge slice is
            # computed on the PE right after its emb is ready, filling the
            # PE idle while the next level's elementwise chain runs.
            _mark(nc, "conductor")
            with tc.tile_pool(name="dwepool", bufs=1) as Pdwe, \
                 tc.tile_pool(name="cps", bufs=2, space="PSUM") as PSc, \
                 tc.tile_pool(name="geps", bufs=2, space="PSUM") as PSg:
                # dwe lives only through the conductor levels (ge fills)
                t_dwe = Pdwe.tile([128, HK, 3 * H], F8, tag="dwe")
                nc.sync.dma_start(t_dwe[:], d["dwe"][:])
                # decoder weights/state needed at dec00 load during the levels
                nc.sync.dma_start(t_h[0][:], d["h0T"][:])
                nc.sync.dma_start(t_c[:], d["c0T"][:])
                nc.sync.dma_start(t_ob[:], d["obias"][:])
                nc.sync.dma_start(t_dbg[:], d["dbg"][:])
                nc.sync.dma_start(t_dwhh[:], d["dwhh"][:])
                nc.sync.dma_start(t_dweg[:], d["dweg"][:])

                def ge_fill(lv):
                    cs = slice(lv * Bc, (lv + 1) * Bc)
                    gp = PSg.tile([128, 3 * HK, Bc], F32, tag="gep",
                                  name="gep")
                    for m in range(3 * HK):
                        nc.tensor.matmul(gp[:, m, :],
                                         t_gebb[0:1, m * 128:(m + 1) * 128],
                                         t_ones[:], start=True, stop=False)
                        for k in range(0, HK, 2):
                            nc.tensor.matmul(
                                gp[:, m, :],
                                t_dwe[:, k:k + 2, m * 128:(m + 1) * 128],
                                t_emb[:, k:k + 2, cs],
                                start=False, stop=(k == HK - 2), perf_mode=DR)
                    nc.vector.tensor_copy(t_ge[:, :, cs], gp[:])

                for _crep in range(COND_REPS):
                  for lv in range(L):
                      cs = slice(lv * Bc, (lv + 1) * Bc)
                      ps_prev = slice((lv - 1) * Bc, lv * Bc)
                      tsig = Pt.tile([128, HK, 3, Bc], BF, tag="tsig",
                                     name="tsig")
                      tg = Pt.tile([128, HK, Bc], BF, tag="tg", name="tg")
                      tcn = Pt.tile([128, HK, Bc], BF, tag="tcn", name="tcn")
                      if lv == 0:
                          # h0 == 0: gates are just gz; c0 == 0
                          nc.scalar.activation(tsig[:], t_gz[:, :, 0:3, cs],
                                               AF.Sigmoid)
                          nc.scalar.activation(tg[:], t_gz[:, :, 3, cs],
                                               AF.Tanh)
                          nc.vector.tensor_mul(t_cc[:], tsig[:, :, 0, :],
                                               tg[:])
                          nc.scalar.activation(tcn[:], t_cc[:], AF.Tanh)
                          nc.vector.tensor_mul(t_emb[:, :, cs],
                                               tsig[:, :, 2, :], tcn[:])
                      else:
                          ps = PSc.tile([128, HK, 4, Bc], F32, tag="cgp",
                                        name="cgp")
                          for p in range(HK):
                              for g in range(4):
                                  ms = slice((p * 4 + g) * 128,
                                             (p * 4 + g + 1) * 128)
                                  for k in range(0, HK, 2):
                                      nc.tensor.matmul(
                                          ps[:, p, g, :],
                                          t_cwhh[:, k:k + 2, ms],
                                          t_emb[:, k:k + 2, ps_prev],
                                          start=(k == 0),
                                          stop=(k == HK - 2), perf_mode=DR)
                          # previous level's ge fills the PE while this
                          # level's elementwise chain runs
                          ge_fill(lv - 1)
                          gs = Pt.tile([128, HK, 4, Bc], BF, tag="gs",
                                       name="gs")
                          tm1 = Pt.tile([128, HK, Bc], BF, tag="tm1",
                                        name="tm1")
                          tm2 = Pt.tile([128, HK, Bc], F32, tag="tm2",
                                        name="tm2")
                          # elementwise in two p-halves, pipelined across
                          # engines; emb half 0 lands early so the next
                          # level's k-outer matmuls can begin
                          for hp in (slice(0, HK // 2), slice(HK // 2, HK)):
                              nc.vector.tensor_add(gs[:, hp, :, :],
                                                   ps[:, hp, :, :],
                                                   t_gz[:, hp, :, cs])
                              nc.scalar.activation(tsig[:, hp, :, :],
                                                   gs[:, hp, 0:3, :],
                                                   AF.Sigmoid)
                              nc.scalar.activation(tg[:, hp, :],
                                                   gs[:, hp, 3, :], AF.Tanh)
                              nc.vector.tensor_mul(tm1[:, hp, :],
                                                   tsig[:, hp, 0, :],
                                                   tg[:, hp, :])
                              nc.gpsimd.tensor_mul(tm2[:, hp, :],
                                                   tsig[:, hp, 1, :],
                                                   t_cc[:, hp, :])
                              nc.vector.tensor_add(t_cc[:, hp, :],
                                                   tm1[:, hp, :],
                                                   tm2[:, hp, :])
                              nc.scalar.activation(tcn[:, hp, :],
                                                   t_cc[:, hp, :], AF.Tanh)
                              nc.vector.tensor_mul(t_emb[:, hp, cs],
                                                   tsig[:, hp, 2, :],
                                                   tcn[:, hp, :])
                  ge_fill(L - 1)

        # remaining decoder weights (first used at dec00 outproj / dec01)
        Pw2 = ctx.enter_context(tc.tile_pool(name="wdec2", bufs=1))
        t_dwn = Pw2.tile([128, TK, 4 * H], F8, tag="dwn")
        nc.sync.dma_start(t_dwn[:], d["dwn"][:])
        t_owt = Pw2.tile([128, HK, T], F8, tag="owt")
        nc.sync.dma_start(t_owt[:], d["owt"][:])

        # ---------------- decoder: 16 note steps over 512 rows --------------
        with tc.tile_pool(name="dtmp", bufs=4) as Pdt, \
             tc.tile_pool(name="dps", bufs=3, space="PSUM") as PSd, \
             tc.tile_pool(name="dpso", bufs=2, space="PSUM") as PSo:
            prefetched = {}
            for _drep in range(DEC_REPS):
              for t in range(NS):
                  _mark(nc, f"dec{t:02d}")
                  hin = t_h[t % 2]
                  hout = t_h[(t + 1) % 2]
                  psAB = {}
                  # software-pipelined emission: stage ops of chunk p are
                  # emitted after stage ops of chunk p+1's predecessors so
                  # each engine's FIFO never head-of-line blocks on a
                  # dependency that a later-emitted independent op could fill.
                  tiles = {}

                  def mms(p):
                      psA = PSd.tile([128, 2, R], F32, tag="dgp", name="psA")
                      if p in prefetched:
                          psB = prefetched.pop(p)
                      else:
                          psB = PSd.tile([128, 2, R], F32, tag="dgp",
                                         name="psB")
                          # g gate: emb contribution recomputed on PE
                          for k in range(0, HK, 2):
                              _mm_dr(nc, psB[:, 1, :], t_dweg, t_emb,
                                     k, slice(p * 128, (p + 1) * 128),
                                     (k == 0), False)
                      psAB[p] = (psA, psB)
                      for gi in range(4):
                          pst = psA if gi < 2 else psB
                          sl = gi % 2
                          ms = slice((p * 4 + gi) * 128,
                                     (p * 4 + gi + 1) * 128)
                          for k in range(0, HK, 2):
                              _mm_dr(nc, pst[:, sl, :], t_dwhh, hin, k, ms,
                                     (k == 0 and gi != 3),
                                     (t == 0 and k == HK - 2))
                          if t > 0:
                              for k in range(0, TK, 2):
                                  _mm_dr(nc, pst[:, sl, :], t_dwn, t_note8,
                                         k, ms, False, (k == TK - 2))

                  FULL = slice(0, R)
                  HALVES = (slice(0, R // 2), slice(R // 2, R))

                  def adds(p, cl=FULL, alloc=True):
                      psA, psB = psAB[p]
                      if alloc:
                          gs3 = Pdt.tile([128, 3, R], BF, tag="gs3",
                                         name="gs3")
                          tg = Pdt.tile([128, R], BF, tag="tg", name="tg")
                          tiles[p] = (gs3, tg)
                      gs3, tg = tiles[p]
                      nc.vector.tensor_add(gs3[:, 2, cl], psB[:, 0, cl],
                                           t_ge[:, 3 * p + 2, cl])
                      nc.vector.tensor_add(gs3[:, 0:2, cl], psA[:, :, cl],
                                           t_ge[:, 3 * p:3 * p + 2, cl])
                      nc.scalar.activation(tg[:, cl], psB[:, 1, cl], AF.Tanh,
                                           bias=t_dbg[:, p:p + 1])

                  def acts(p, cl=FULL, alloc=True):
                      if alloc:
                          gs3, tg = tiles[p]
                          tsig = Pdt.tile([128, 3, R], BF, tag="tsig3",
                                          name="tsig3")
                          tiles[p] = (tsig, tg, gs3)
                      tsig, tg, gs3 = tiles[p]
                      nc.scalar.activation(tsig[:, :, cl], gs3[:, :, cl],
                                           AF.Sigmoid)

                  def tail(p, cl=FULL, alloc=True):
                      tsig, tg = tiles[p][0], tiles[p][1]
                      if alloc:
                          tcn = Pdt.tile([128, R], BF, tag="tcn", name="tcn")
                          tm1 = Pdt.tile([128, R], BF, tag="tm1", name="tm1")
                          tm2 = Pdt.tile([128, R], F32, tag="tm2", name="tm2")
                          tiles[(p, 'x')] = (tcn, tm1, tm2)
                      tcn, tm1, tm2 = tiles[(p, 'x')]
                      nc.vector.tensor_mul(tm1[:, cl], tsig[:, 0, cl],
                                           tg[:, cl])
                      nc.gpsimd.tensor_mul(tm2[:, cl], tsig[:, 1, cl],
                                           t_c[:, p, cl])
                      ceng = nc.vector if p >= HK - 2 else nc.gpsimd
                      ceng.tensor_add(t_c[:, p, cl], tm1[:, cl],
                                      tm2[:, cl])
                      nc.scalar.activation(tcn[:, cl], t_c[:, p, cl], AF.Tanh)
                      nc.vector.tensor_mul(hout[:, p, cl], tsig[:, 2, cl],
                                           tcn[:, cl])

                  for p in range(HK + 3):
                      if p < HK:
                          mms(p)
                          if p < HK - 2:
                              adds(p)
                          else:
                              # last two chunks: half-R ops so the
                              # step-boundary chain pipelines at finer grain
                              adds(p, HALVES[0])
                              adds(p, HALVES[1], alloc=False)
                      if 1 <= p and p - 1 < HK:
                          if p - 1 < HK - 2:
                              acts(p - 1)
                          else:
                              acts(p - 1, HALVES[0])
                              acts(p - 1, HALVES[1], alloc=False)
                      if p >= 3:
                          q = p - 3
                          if q < HK - 2:
                              tail(q)
                          else:
                              tail(q, HALVES[0])
                              tail(q, HALVES[1], alloc=False)
                  # output projection + sigmoid -> note (fp8 mirror first so
                  # the next step's Wn matmuls unblock as early as possible)
                  for tk in range(TK):
                      ts_ = slice(tk * 128, (tk + 1) * 128)
                      po = PSo.tile([128, R], F32, tag="dpo", name="dpo")
                      for k in range(0, HK, 2):
                          _mm_dr(nc, po[:], t_owt, hout, k, ts_,
                                 (k == 0), (k == HK - 2))
                      nc.scalar.activation(t_note[:, tk, :], po[:],
                                           AF.Sigmoid, bias=t_ob[:, tk:tk + 1])
                      if t + 1 < NS:
                          nc.vector.tensor_copy(t_note8[:, tk, :],
                                                t_note[:, tk, :])
                      nc.sync.dma_start(d["outbuf"][t, tk], t_note[:, tk, :])


import os
DEC_REPS = int(os.environ.get("KBENCH_DEC_REPS", "1"))
COND_REPS = int(os.environ.get("KBENCH_COND_REPS", "1"))

_CACHE = {}


def _build():
    if "nc" not in _CACHE:
        nc = bacc.Bacc("TRN2", target_bir_lowering=False, debug=False,
                       num_devices=NCORES)
        d = _declare(nc)
        with tile.TileContext(nc) as tc:
            _body(nc, tc, d)
        nc.compile()
        _CACHE["nc"] = nc
    return _CACHE["nc"]


def _feat_major(W, dt):
    """[J, K] -> [128, K/128, J] (stationary lhsT chunk layout)."""
    J, K = W.shape
    return np.ascontiguousarray(
        W.reshape(J, K // 128, 128).transpose(2, 1, 0)).astype(dt)


def _reorder4(W):
    """[4H, K] rows in PyTorch gate blocks (i,f,g,o) -> p-adjacent chunks in
    order (i,f,o,g): new chunk m = p*4 + {0:i,1:f,2:o,3:g}."""
    K = W.shape[1]
    W4 = W.reshape(4, HK, 128, K)[[0, 1, 3, 2]]
    return np.ascontiguousarray(W4.transpose(1, 0, 2, 3).reshape(4 * H, K))


def _pack_inputs(inputs):
    z = np.asarray(inputs["z"], np.float32)
    dec_h0 = np.asarray(inputs["dec_h0"], np.float32)
    dec_c0 = np.asarray(inputs["dec_c0"], np.float32)
    cond_b = np.asarray(inputs["cond_bih"] + inputs["cond_bhh"], np.float32)
    dec_b = np.asarray(inputs["dec_bih"] + inputs["dec_bhh"], np.float32)
    out_b = np.asarray(inputs["out_b"], np.float32)

    cb4 = cond_b.reshape(4, HK, 128)[[0, 1, 3, 2]]
    cbt = np.ascontiguousarray(cb4.transpose(1, 0, 2).reshape(G, 128).T)
    db4 = dec_b.reshape(4, HK, 128)
    gebb = np.ascontiguousarray(
        db4[[0, 1, 3]].transpose(1, 0, 2).reshape(1, 3 * H))
    dbg = np.ascontiguousarray(db4[2].T)

    We = np.asarray(inputs["dec_Wih"][:, :H], np.float32)
    We4 = We.reshape(4, HK, 128, H)
    We3 = np.ascontiguousarray(
        We4[[0, 1, 3]].transpose(1, 0, 2, 3).reshape(3 * H, H))
    dwe = _feat_major(We3, f8)                      # [128, HK, 3H]
    dweg = _feat_major(np.ascontiguousarray(We4[2].reshape(H, H)), f8)

    shared = {
        "cbt": cbt.astype(np.float32),
        "dbg": dbg.astype(np.float32),
        "ones": np.ones((1, Bc), dtype=bf16),
        "gebb": gebb.astype(bf16),
        "obias": np.ascontiguousarray(out_b.reshape(TK, 128).T).astype(np.float32),
        "cwih": _feat_major(_reorder4(np.asarray(inputs["cond_Wih"], np.float32)), f8),
        "cwhh": _feat_major(_reorder4(np.asarray(inputs["cond_Whh"], np.float32)), f8),
        "dwn": _feat_major(_reorder4(np.asarray(inputs["dec_Wih"][:, H:], np.float32)), f8),
        "dwhh": _feat_major(_reorder4(np.asarray(inputs["dec_Whh"], np.float32)), f8),
        "owt": _feat_major(np.asarray(inputs["out_W"], np.float32), f8),
        "dwe": dwe,
        "dweg": dweg,
    }

    z_lv = z[:, np.arange(L) * L, 0, :]           # [B, L, Z]
    in_maps = []
    for c in range(NCORES):
        bs = slice(c * Bc, (c + 1) * Bc)
        zc = z_lv[bs]                              # [Bc, L, Z]
        zT = np.ascontiguousarray(
            zc.reshape(Bc, L, ZK, 128).transpose(3, 2, 1, 0).reshape(128, ZK, R)
        ).astype(f8)
        h0 = dec_h0[:, bs, :]                      # [L, Bc, H]
        h0T = np.ascontiguousarray(
            h0.reshape(L, Bc, HK, 128).transpose(3, 2, 0, 1).reshape(128, HK, R))
        c0 = dec_c0[:, bs, :]
        c0T = np.ascontiguousarray(
            c0.reshape(L, Bc, HK, 128).transpose(3, 2, 0, 1).reshape(128, HK, R))
        m = dict(shared)
        m["zT"] = zT
        m["h0T"] = h0T.astype(f8)
        m["c0T"] = c0T.astype(np.float32)
        in_maps.append(m)
    return in_maps


def _unpack_outputs(core_outs):
    notes = np.empty((B, L * NS, T), np.float32)
    for c, arr in enumerate(core_outs):
        # arr [NS, TK, 128, R] -> [Bc, L, NS, T]
        a = arr.astype(np.float32).reshape(NS, TK, 128, L, Bc).transpose(4, 3, 0, 1, 2)
        notes[c * Bc:(c + 1) * Bc] = a.reshape(Bc, L, NS, T).reshape(
            Bc, L * NS, T)
    return notes


def kernel(**inputs):
    nc = _build()
    in_maps = _pack_inputs(inputs)
    res = run_bass_kernel_spmd(nc, in_maps, list(range(NCORES)))
    return _unpack_outputs([r["outbuf"] for r in res.results])


# revision 55
# speedup vs baseline: 1.0455x; 1.0005x over previous
"""Trainium2 Bass kernel for nn_Decoder (MusicVAE-style hierarchical decoder).

Strategy (8 NeuronCores, data-parallel over batch, no inter-core comms):
  - Conductor LSTM (16 sequential levels, batch 32/core) computes per-level
    embeddings; decoder levels are independent, so all 16 levels are batched:
    512 decoder rows per core, 16 sequential note steps.
  - fp8(e4m3) matmuls, DoubleRow perf mode for 512-row streams; fp32 PSUM.
  - Gate chunks are laid out p-adjacent in order (i, f, o, g) so the three
    sigmoid gates batch into one activation op and (i,f) / (o,g) pairs map
    onto two 2-bank PSUM accumulation tiles.
  - The g-gate's conductor-embedding contribution (emb @ dec_Wih_g.T) is
    recomputed on the PE every step (cheaper than a DVE add at model rates);
    its bias rides the tanh activation's bias port.  i/f/o biases are folded
    into ge / gz via Identity-activation copies (no bias matmuls, no ones).
  - Elementwise work is spread across DVE (vector), Pool (gpsimd) and
    Activation engines to balance the per-step makespan against the PE.
"""
import numpy as np
import ml_dtypes

import concourse.bacc as bacc
import concourse.tile as tile
import concourse.mybir as mybir
from concourse.bass_utils import run_bass_kernel_spmd

bf16 = ml_dtypes.bfloat16
f8 = ml_dtypes.float8_e4m3
F32 = mybir.dt.float32
BF = mybir.dt.bfloat16
F8 = mybir.dt.float8e4
AF = mybir.ActivationFunctionType
DR = mybir.MatmulPerfMode.DoubleRow

NCORES = 8
B, Z, H, T = 256, 512, 1024, 512
L, NS = 16, 16
Bc = B // NCORES            # 32 batch rows per core
R = L * Bc                  # 512 decoder rows per core (levels x batch)
HK, TK, ZK = H // 128, T // 128, Z // 128   # 8, 4, 4
G = 4 * H // 128            # 32 gate chunks of 128


def _declare(nc):
    d = {}
    ei = dict(kind="ExternalInput")
    d["cbt"] = nc.dram_tensor("cbt", [128, G], F32, **ei)
    d["dbg"] = nc.dram_tensor("dbg", [128, HK], F32, **ei)
    d["obias"] = nc.dram_tensor("obias", [128, TK], F32, **ei)
    d["zT"] = nc.dram_tensor("zT", [128, ZK, R], F8, **ei)
    d["h0T"] = nc.dram_tensor("h0T", [128, HK, R], F8, **ei)
    d["c0T"] = nc.dram_tensor("c0T", [128, HK, R], F32, **ei)
    d["cwih"] = nc.dram_tensor("cwih", [128, ZK, 4 * H], F8, **ei)
    d["cwhh"] = nc.dram_tensor("cwhh", [128, HK, 4 * H], F8, **ei)
    d["ones"] = nc.dram_tensor("ones", [1, Bc], BF, **ei)
    d["gebb"] = nc.dram_tensor("gebb", [1, 3 * H], BF, **ei)
    d["dwe"] = nc.dram_tensor("dwe", [128, HK, 3 * H], F8, **ei)
    d["dweg"] = nc.dram_tensor("dweg", [128, HK, H], F8, **ei)
    d["dwn"] = nc.dram_tensor("dwn", [128, TK, 4 * H], F8, **ei)
    d["dwhh"] = nc.dram_tensor("dwhh", [128, HK, 4 * H], F8, **ei)
    d["owt"] = nc.dram_tensor("owt", [128, HK, T], F8, **ei)
    d["outbuf"] = nc.dram_tensor("outbuf", [NS, TK, 128, R], BF,
                                 kind="ExternalOutput")
    return d


def _mm_dr(nc, out, w3, x3, ks, ms, start, stop):
    """DoubleRow fp8 matmul over k-subtile pair (ks, ks+1)."""
    return nc.tensor.matmul(out, w3[:, ks:ks + 2, ms], x3[:, ks:ks + 2, :],
                            start=start, stop=stop, perf_mode=DR)


PHASE_MARKS = []


def _mark(nc, name):
    try:
        PHASE_MARKS.append((name, sum(1 for _ in nc.all_instructions())))
    except Exception:
        pass


def _body(nc, tc, d):
    import contextlib
    with contextlib.ExitStack() as ctx:
        Pp = ctx.enter_context(tc.tile_pool(name="persist", bufs=1))

        t_ob = Pp.tile([128, TK], F32, tag="obias")
        t_dbg = Pp.tile([128, HK], F32, tag="dbg")
        t_emb = Pp.tile([128, HK, R], F8, tag="emb")
        t_h = [Pp.tile([128, HK, R], F8, tag=f"hT{i}", name=f"hT{i}")
               for i in (0, 1)]
        t_c = Pp.tile([128, HK, R], F32, tag="c")
        t_note = Pp.tile([128, TK, R], BF, tag="note")
        t_note8 = Pp.tile([128, TK, R], F8, tag="note8")
        # ge persists through the decoder; filled per-level in the conductor
        t_ge = Pp.tile([128, 3 * HK, R], BF, tag="ge")
        # decoder weights needed at dec00 (loaded during the conductor)
        t_dwhh = Pp.tile([128, HK, 4 * H], F8, tag="dwhh")
        t_dweg = Pp.tile([128, HK, H], F8, tag="dweg")

        # ---------------- conductor (+ per-level ge fill) ----------------
        with tc.tile_pool(name="cond", bufs=1) as Pc, \
             tc.tile_pool(name="ctmp", bufs=2) as Pt:
            # DMA order on the shared engine: gz deps first, then cwhh
            # (needed at level 1), then dwe (ge fills), then the rest.
            t_cwhh = Pc.tile([128, HK, 4 * H], F8, tag="cwhh")
            t_ones = Pc.tile([1, Bc], BF, tag="ones")
            t_gebb = Pc.tile([1, 3 * H], BF, tag="gebb")
            # gz chunks indexed [p, gate] with gate order (i, f, o, g)
            t_gz = Pc.tile([128, HK, 4, R], BF, tag="gz")
            t_cc = Pc.tile([128, HK, Bc], F32, tag="cc")

            # gz = z @ cond_Wih.T + cond_b for all levels at once (fp8 DR);
            # bias folded via Identity-activation copy from PSUM.
            _mark(nc, "gz")
            with tc.tile_pool(name="condz", bufs=1) as Pcz, \
                 tc.tile_pool(name="gzps", bufs=2, space="PSUM") as PSz:
                t_cwih = Pcz.tile([128, ZK, 4 * H], F8, tag="cwih")
                t_zT = Pcz.tile([128, ZK, R], F8, tag="zT")
                nc.sync.dma_start(t_zT[:], d["zT"][:])
                t_cbt = Pcz.tile([128, G], F32, tag="cbt")
                nc.sync.dma_start(t_cbt[:], d["cbt"][:])
                # cwih in halves so the first gz chunks start ~4us earlier
                half = 2 * H
                nc.sync.dma_start(t_cwih[:, :, 0:half],
                                  d["cwih"][:, :, 0:half])
                nc.sync.dma_start(t_cwih[:, :, half:4 * H],
                                  d["cwih"][:, :, half:4 * H])
                nc.sync.dma_start(t_cwhh[:], d["cwhh"][:])
                nc.sync.dma_start(t_ones[:], d["ones"][:])
                nc.sync.dma_start(t_gebb[:], d["gebb"][:])
                for m in range(G):
                    ms = slice(m * 128, (m + 1) * 128)
                    ps = PSz.tile([128, R], F32, tag="gzp", name="gzp")
                    for k in range(0, ZK, 2):
                        _mm_dr(nc, ps[:], t_cwih, t_zT, k, ms,
                               (k == 0), (k == ZK - 2))
                    nc.scalar.activation(t_gz[:, m // 4, m % 4, :], ps[:],
                                         AF.Identity, bias=t_cbt[:, m:m + 1])

            # sequential levels (fp8 non-DR: 32-row streams), elementwise
            # batched level-wide via strided views; each level's ge slice is
            # computed on the PE right after its emb is ready, filling the
            # PE idle while the next level's elementwise chain runs.
            _mark(nc, "conductor")
            with tc.tile_pool(name="dwepool", bufs=1) as Pdwe, \
                 tc.tile_pool(name="cps", bufs=2, space="PSUM") as PSc, \
                 tc.tile_pool(name="geps", bufs=2, space="PSUM") as PSg:
                # dwe lives only through the conductor levels (ge fills)
                t_dwe = Pdwe.tile([128, HK, 3 * H], F8, tag="dwe")
                nc.sync.dma_start(t_dwe[:], d["dwe"][:])
                # decoder weights/state needed at dec00 load during the levels
                nc.sync.dma_start(t_h[0][:], d["h0T"][:])
                nc.sync.dma_start(t_c[:], d["c0T"][:])
                nc.sync.dma_start(t_ob[:], d["obias"][:])
                nc.sync.dma_start(t_dbg[:], d["dbg"][:])
                nc.sync.dma_start(t_dwhh[:], d["dwhh"][:])
                nc.sync.dma_start(t_dweg[:], d["dweg"][:])

                def ge_fill(lv):
                    cs = slice(lv * Bc, (lv + 1) * Bc)
                    gp = PSg.tile([128, 3 * HK, Bc], F32, tag="gep",
                                  name="gep")
                    for m in range(3 * HK):
                        nc.tensor.matmul(gp[:, m, :],
                                         t_gebb[0:1, m * 128:(m + 1) * 128],
                                         t_ones[:], start=True, stop=False)
                        for k in range(0, HK, 2):
                            nc.tensor.matmul(
                                gp[:, m, :],
                                t_dwe[:, k:k + 2, m * 128:(m + 1) * 128],
                                t_emb[:, k:k + 2, cs],
                                start=False, stop=(k == HK - 2), perf_mode=DR)
                    nc.vector.tensor_copy(t_ge[:, :, cs], gp[:])

                for _crep in range(COND_REPS):
                  for lv in range(L):
                      cs = slice(lv * Bc, (lv + 1) * Bc)
                      ps_prev = slice((lv - 1) * Bc, lv * Bc)
                      tsig = Pt.tile([128, HK, 3, Bc], BF, tag="tsig",
                                     name="tsig")
                      tg = Pt.tile([128, HK, Bc], BF, tag="tg", name="tg")
                      tcn = Pt.tile([128, HK, Bc], BF, tag="tcn", name="tcn")
                      if lv == 0:
                          # h0 == 0: gates are just gz; c0 == 0
                          nc.scalar.activation(tsig[:], t_gz[:, :, 0:3, cs],
                                               AF.Sigmoid)
                          nc.scalar.activation(tg[:], t_gz[:, :, 3, cs],
                                               AF.Tanh)
                          nc.vector.tensor_mul(t_cc[:], tsig[:, :, 0, :],
                                               tg[:])
                          nc.scalar.activation(tcn[:], t_cc[:], AF.Tanh)
                          nc.vector.tensor_mul(t_emb[:, :, cs],
                                               tsig[:, :, 2, :], tcn[:])
                      else:
                          ps = PSc.tile([128, HK, 4, Bc], F32, tag="cgp",
                                        name="cgp")
                          for p in range(HK):
                              for g in range(4):
                                  ms = slice((p * 4 + g) * 128,
                                             (p * 4 + g + 1) * 128)
                                  for k in range(0, HK, 2):
                                      nc.tensor.matmul(
                                          ps[:, p, g, :],
                                          t_cwhh[:, k:k + 2, ms],
                                          t_emb[:, k:k + 2, ps_prev],
                                          start=(k == 0),
                                          stop=(k == HK - 2), perf_mode=DR)
                          # ge fill lagged two levels: keeps it off the
                          # dwe-DMA critical window at conductor start
                          if lv >= 2:
                              ge_fill(lv - 2)
                          gs = Pt.tile([128, HK, 4, Bc], BF, tag="gs",
                                       name="gs")
                          tm1 = Pt.tile([128, HK, Bc], BF, tag="tm1",
                                        name="tm1")
                          tm2 = Pt.tile([128, HK, Bc], F32, tag="tm2",
                                        name="tm2")
                          # elementwise in two p-halves, pipelined across
                          # engines; emb half 0 lands early so the next
                          # level's k-outer matmuls can begin
                          for hp in (slice(0, HK // 2), slice(HK // 2, HK)):
                              nc.vector.tensor_add(gs[:, hp, :, :],
                                                   ps[:, hp, :, :],
                                                   t_gz[:, hp, :, cs])
                              nc.scalar.activation(tsig[:, hp, :, :],
                                                   gs[:, hp, 0:3, :],
                                                   AF.Sigmoid)
                              nc.scalar.activation(tg[:, hp, :],
                                                   gs[:, hp, 3, :], AF.Tanh)
                              nc.vector.tensor_mul(tm1[:, hp, :],
                                                   tsig[:, hp, 0, :],
                                                   tg[:, hp, :])
                              nc.gpsimd.tensor_mul(tm2[:, hp, :],
                                                   tsig[:, hp, 1, :],
                                                   t_cc[:, hp, :])
                              nc.vector.tensor_add(t_cc[:, hp, :],
                                                   tm1[:, hp, :],
                                                   tm2[:, hp, :])
                              nc.scalar.activation(tcn[:, hp, :],
                                                   t_cc[:, hp, :], AF.Tanh)
                              nc.vector.tensor_mul(t_emb[:, hp, cs],
                                                   tsig[:, hp, 2, :],
                                                   tcn[:, hp, :])
                  ge_fill(L - 2)
                  ge_fill(L - 1)

        # remaining decoder weights (first used at dec00 outproj / dec01)
        Pw2 = ctx.enter_context(tc.tile_pool(name="wdec2", bufs=1))
        t_dwn = Pw2.tile([128, TK, 4 * H], F8, tag="dwn")
        nc.sync.dma_start(t_dwn[:], d["dwn"][:])
        t_owt = Pw2.tile([128, HK, T], F8, tag="owt")
        nc.sync.dma_start(t_owt[:], d["owt"][:])

        # ---------------- decoder: 16 note steps over 512 rows --------------
        with tc.tile_pool(name="dtmp", bufs=4) as Pdt, \
             tc.tile_pool(name="dps", bufs=3, space="PSUM") as PSd, \
             tc.tile_pool(name="dpso", bufs=2, space="PSUM") as PSo:
            prefetched = {}
            for _drep in range(DEC_REPS):
              for t in range(NS):
                  _mark(nc, f"dec{t:02d}")
                  hin = t_h[t % 2]
                  hout = t_h[(t + 1) % 2]
                  psAB = {}
                  # software-pipelined emission: stage ops of chunk p are
                  # emitted after stage ops of chunk p+1's predecessors so
                  # each engine's FIFO never head-of-line blocks on a
                  # dependency that a later-emitted independent op could fill.
                  tiles = {}

                  def mms(p):
                      psA = PSd.tile([128, 2, R], F32, tag="dgp", name="psA")
                      if p in prefetched:
                          psB = prefetched.pop(p)
                      else:
                          psB = PSd.tile([128, 2, R], F32, tag="dgp",
                                         name="psB")
                          # g gate: emb contribution recomputed on PE
                          for k in range(0, HK, 2):
                              _mm_dr(nc, psB[:, 1, :], t_dweg, t_emb,
                                     k, slice(p * 128, (p + 1) * 128),
                                     (k == 0), False)
                      psAB[p] = (psA, psB)
                      for gi in range(4):
                          pst = psA if gi < 2 else psB
                          sl = gi % 2
                          ms = slice((p * 4 + gi) * 128,
                                     (p * 4 + gi + 1) * 128)
                          for k in range(0, HK, 2):
                              _mm_dr(nc, pst[:, sl, :], t_dwhh, hin, k, ms,
                                     (k == 0 and gi != 3),
                                     (t == 0 and k == HK - 2))
                          if t > 0:
                              for k in range(0, TK, 2):
                                  _mm_dr(nc, pst[:, sl, :], t_dwn, t_note8,
                                         k, ms, False, (k == TK - 2))

                  FULL = slice(0, R)
                  HALVES = (slice(0, R // 2), slice(R // 2, R))

                  def adds(p, cl=FULL, alloc=True):
                      psA, psB = psAB[p]
                      if alloc:
                          gs3 = Pdt.tile([128, 3, R], BF, tag="gs3",
                                         name="gs3")
                          tg = Pdt.tile([128, R], BF, tag="tg", name="tg")
                          tiles[p] = (gs3, tg)
                      gs3, tg = tiles[p]
                      nc.vector.tensor_add(gs3[:, 2, cl], psB[:, 0, cl],
                                           t_ge[:, 3 * p + 2, cl])
                      nc.vector.tensor_add(gs3[:, 0:2, cl], psA[:, :, cl],
                                           t_ge[:, 3 * p:3 * p + 2, cl])
                      nc.scalar.activation(tg[:, cl], psB[:, 1, cl], AF.Tanh,
                                           bias=t_dbg[:, p:p + 1])

                  def acts(p, cl=FULL, alloc=True):
                      if alloc:
                          gs3, tg = tiles[p]
                          tsig = Pdt.tile([128, 3, R], BF, tag="tsig3",
                                          name="tsig3")
                          tiles[p] = (tsig, tg, gs3)
                      tsig, tg, gs3 = tiles[p]
                      nc.scalar.activation(tsig[:, :, cl], gs3[:, :, cl],
                                           AF.Sigmoid)

                  def tail(p, cl=FULL, alloc=True):
                      tsig, tg = tiles[p][0], tiles[p][1]
                      if alloc:
                          tcn = Pdt.tile([128, R], BF, tag="tcn", name="tcn")
                          tm1 = Pdt.tile([128, R], BF, tag="tm1", name="tm1")
                          tm2 = Pdt.tile([128, R], F32, tag="tm2", name="tm2")
                          tiles[(p, 'x')] = (tcn, tm1, tm2)
                      tcn, tm1, tm2 = tiles[(p, 'x')]
                      nc.vector.tensor_mul(tm1[:, cl], tsig[:, 0, cl],
                                           tg[:, cl])
                      nc.gpsimd.tensor_mul(tm2[:, cl], tsig[:, 1, cl],
                                           t_c[:, p, cl])
                      ceng = nc.vector if p >= HK - 2 else nc.gpsimd
                      ceng.tensor_add(t_c[:, p, cl], tm1[:, cl],
                                      tm2[:, cl])
                      nc.scalar.activation(tcn[:, cl], t_c[:, p, cl], AF.Tanh)
                      nc.vector.tensor_mul(hout[:, p, cl], tsig[:, 2, cl],
                                           tcn[:, cl])

                  for p in range(HK + 3):
                      if p < HK:
                          mms(p)
                          if p < HK - 2:
                              adds(p)
                          else:
                              # last two chunks: half-R ops so the
                              # step-boundary chain pipelines at finer grain
                              adds(p, HALVES[0])
                              adds(p, HALVES[1], alloc=False)
                      if 1 <= p and p - 1 < HK:
                          if p - 1 < HK - 2:
                              acts(p - 1)
                          else:
                              acts(p - 1, HALVES[0])
                              acts(p - 1, HALVES[1], alloc=False)
                      if p >= 3:
                          q = p - 3
                          if q < HK - 2:
                              tail(q)
                          else:
                              tail(q, HALVES[0])
                              tail(q, HALVES[1], alloc=False)
                  # output projection + sigmoid -> note (fp8 mirror first so
                  # the next step's Wn matmuls unblock as early as possible)
                  for tk in range(TK):
                      ts_ = slice(tk * 128, (tk + 1) * 128)
                      po = PSo.tile([128, R], F32, tag="dpo", name="dpo")
                      for k in range(0, HK, 2):
                          _mm_dr(nc, po[:], t_owt, hout, k, ts_,
                                 (k == 0), (k == HK - 2))
                      nc.scalar.activation(t_note[:, tk, :], po[:],
                                           AF.Sigmoid, bias=t_ob[:, tk:tk + 1])
                      if t + 1 < NS:
                          nc.vector.tensor_copy(t_note8[:, tk, :],
                                                t_note[:, tk, :])
                      nc.sync.dma_start(d["outbuf"][t, tk], t_note[:, tk, :])


import os
DEC_REPS = int(os.environ.get("KBENCH_DEC_REPS", "1"))
COND_REPS = int(os.environ.get("KBENCH_COND_REPS", "1"))

_CACHE = {}


def _build():
    if "nc" not in _CACHE:
        nc = bacc.Bacc("TRN2", target_bir_lowering=False, debug=False,
                       num_devices=NCORES)
        d = _declare(nc)
        with tile.TileContext(nc) as tc:
            _body(nc, tc, d)
        nc.compile()
        _CACHE["nc"] = nc
    return _CACHE["nc"]


def _feat_major(W, dt):
    """[J, K] -> [128, K/128, J] (stationary lhsT chunk layout)."""
    J, K = W.shape
    return np.ascontiguousarray(
        W.reshape(J, K // 128, 128).transpose(2, 1, 0)).astype(dt)


def _reorder4(W):
    """[4H, K] rows in PyTorch gate blocks (i,f,g,o) -> p-adjacent chunks in
    order (i,f,o,g): new chunk m = p*4 + {0:i,1:f,2:o,3:g}."""
    K = W.shape[1]
    W4 = W.reshape(4, HK, 128, K)[[0, 1, 3, 2]]
    return np.ascontiguousarray(W4.transpose(1, 0, 2, 3).reshape(4 * H, K))


def _pack_inputs(inputs):
    z = np.asarray(inputs["z"], np.float32)
    dec_h0 = np.asarray(inputs["dec_h0"], np.float32)
    dec_c0 = np.asarray(inputs["dec_c0"], np.float32)
    cond_b = np.asarray(inputs["cond_bih"] + inputs["cond_bhh"], np.float32)
    dec_b = np.asarray(inputs["dec_bih"] + inputs["dec_bhh"], np.float32)
    out_b = np.asarray(inputs["out_b"], np.float32)

    cb4 = cond_b.reshape(4, HK, 128)[[0, 1, 3, 2]]
    cbt = np.ascontiguousarray(cb4.transpose(1, 0, 2).reshape(G, 128).T)
    db4 = dec_b.reshape(4, HK, 128)
    gebb = np.ascontiguousarray(
        db4[[0, 1, 3]].transpose(1, 0, 2).reshape(1, 3 * H))
    dbg = np.ascontiguousarray(db4[2].T)

    We = np.asarray(inputs["dec_Wih"][:, :H], np.float32)
    We4 = We.reshape(4, HK, 128, H)
    We3 = np.ascontiguousarray(
        We4[[0, 1, 3]].transpose(1, 0, 2, 3).reshape(3 * H, H))
    dwe = _feat_major(We3, f8)                      # [128, HK, 3H]
    dweg = _feat_major(np.ascontiguousarray(We4[2].reshape(H, H)), f8)

    shared = {
        "cbt": cbt.astype(np.float32),
        "dbg": dbg.astype(np.float32),
        "ones": np.ones((1, Bc), dtype=bf16),
        "gebb": gebb.astype(bf16),
        "obias": np.ascontiguousarray(out_b.reshape(TK, 128).T).astype(np.float32),
        "cwih": _feat_major(_reorder4(np.asarray(inputs["cond_Wih"], np.float32)), f8),
        "cwhh": _feat_major(_reorder4(np.asarray(inputs["cond_Whh"], np.float32)), f8),
        "dwn": _feat_major(_reorder4(np.asarray(inputs["dec_Wih"][:, H:], np.float32)), f8),
        "dwhh": _feat_major(_reorder4(np.asarray(inputs["dec_Whh"], np.float32)), f8),
        "owt": _feat_major(np.asarray(inputs["out_W"], np.float32), f8),
        "dwe": dwe,
        "dweg": dweg,
    }

    z_lv = z[:, np.arange(L) * L, 0, :]           # [B, L, Z]
    in_maps = []
    for c in range(NCORES):
        bs = slice(c * Bc, (c + 1) * Bc)
        zc = z_lv[bs]                              # [Bc, L, Z]
        zT = np.ascontiguousarray(
            zc.reshape(Bc, L, ZK, 128).transpose(3, 2, 1, 0).reshape(128, ZK, R)
        ).astype(f8)
        h0 = dec_h0[:, bs, :]                      # [L, Bc, H]
        h0T = np.ascontiguousarray(
            h0.reshape(L, Bc, HK, 128).transpose(3, 2, 0, 1).reshape(128, HK, R))
        c0 = dec_c0[:, bs, :]
        c0T = np.ascontiguousarray(
            c0.reshape(L, Bc, HK, 128).transpose(3, 2, 0, 1).reshape(128, HK, R))
        m = dict(shared)
        m["zT"] = zT
        m["h0T"] = h0T.astype(f8)
        m["c0T"] = c0T.astype(np.float32)
        in_maps.append(m)
    return in_maps


def _unpack_outputs(core_outs):
    notes = np.empty((B, L * NS, T), np.float32)
    for c, arr in enumerate(core_outs):
        # arr [NS, TK, 128, R] -> [Bc, L, NS, T]
        a = arr.astype(np.float32).reshape(NS, TK, 128, L, Bc).transpose(4, 3, 0, 1, 2)
        notes[c * Bc:(c + 1) * Bc] = a.reshape(Bc, L, NS, T).reshape(
            Bc, L * NS, T)
    return notes


def kernel(**inputs):
    nc = _build()
    in_maps = _pack_inputs(inputs)
    res = run_bass_kernel_spmd(nc, in_maps, list(range(NCORES)))
    return _unpack_outputs([r["outbuf"] for r in res.results])


# revision 56
# speedup vs baseline: 1.0475x; 1.0020x over previous
"""Trainium2 Bass kernel for nn_Decoder (MusicVAE-style hierarchical decoder).

Strategy (8 NeuronCores, data-parallel over batch, no inter-core comms):
  - Conductor LSTM (16 sequential levels, batch 32/core) computes per-level
    embeddings; decoder levels are independent, so all 16 levels are batched:
    512 decoder rows per core, 16 sequential note steps.
  - fp8(e4m3) matmuls, DoubleRow perf mode for 512-row streams; fp32 PSUM.
  - Gate chunks are laid out p-adjacent in order (i, f, o, g) so the three
    sigmoid gates batch into one activation op and (i,f) / (o,g) pairs map
    onto two 2-bank PSUM accumulation tiles.
  - The g-gate's conductor-embedding contribution (emb @ dec_Wih_g.T) is
    recomputed on the PE every step (cheaper than a DVE add at model rates);
    its bias rides the tanh activation's bias port.  i/f/o biases are folded
    into ge / gz via Identity-activation copies (no bias matmuls, no ones).
  - Elementwise work is spread across DVE (vector), Pool (gpsimd) and
    Activation engines to balance the per-step makespan against the PE.
"""
import numpy as np
import ml_dtypes

import concourse.bacc as bacc
import concourse.tile as tile
import concourse.mybir as mybir
from concourse.bass_utils import run_bass_kernel_spmd

bf16 = ml_dtypes.bfloat16
f8 = ml_dtypes.float8_e4m3
F32 = mybir.dt.float32
BF = mybir.dt.bfloat16
F8 = mybir.dt.float8e4
AF = mybir.ActivationFunctionType
DR = mybir.MatmulPerfMode.DoubleRow

NCORES = 8
B, Z, H, T = 256, 512, 1024, 512
L, NS = 16, 16
Bc = B // NCORES            # 32 batch rows per core
R = L * Bc                  # 512 decoder rows per core (levels x batch)
HK, TK, ZK = H // 128, T // 128, Z // 128   # 8, 4, 4
G = 4 * H // 128            # 32 gate chunks of 128


def _declare(nc):
    d = {}
    ei = dict(kind="ExternalInput")
    d["cbt"] = nc.dram_tensor("cbt", [128, G], F32, **ei)
    d["dbg"] = nc.dram_tensor("dbg", [128, HK], F32, **ei)
    d["obias"] = nc.dram_tensor("obias", [128, TK], F32, **ei)
    d["zT"] = nc.dram_tensor("zT", [128, ZK, R], F8, **ei)
    d["h0T"] = nc.dram_tensor("h0T", [128, HK, R], F8, **ei)
    d["c0T"] = nc.dram_tensor("c0T", [128, HK, R], F32, **ei)
    d["cwih"] = nc.dram_tensor("cwih", [128, ZK, 4 * H], F8, **ei)
    d["cwhh"] = nc.dram_tensor("cwhh", [128, HK, 4 * H], F8, **ei)
    d["ones"] = nc.dram_tensor("ones", [1, Bc], BF, **ei)
    d["gebb"] = nc.dram_tensor("gebb", [1, 3 * H], BF, **ei)
    d["dwe"] = nc.dram_tensor("dwe", [128, HK, 3 * H], F8, **ei)
    d["dweg"] = nc.dram_tensor("dweg", [128, HK, H], F8, **ei)
    d["dwn"] = nc.dram_tensor("dwn", [128, TK, 4 * H], F8, **ei)
    d["dwhh"] = nc.dram_tensor("dwhh", [128, HK, 4 * H], F8, **ei)
    d["owt"] = nc.dram_tensor("owt", [128, HK, T], F8, **ei)
    d["outbuf"] = nc.dram_tensor("outbuf", [NS, TK, 128, R], BF,
                                 kind="ExternalOutput")
    return d


def _mm_dr(nc, out, w3, x3, ks, ms, start, stop):
    """DoubleRow fp8 matmul over k-subtile pair (ks, ks+1)."""
    return nc.tensor.matmul(out, w3[:, ks:ks + 2, ms], x3[:, ks:ks + 2, :],
                            start=start, stop=stop, perf_mode=DR)


PHASE_MARKS = []


def _mark(nc, name):
    try:
        PHASE_MARKS.append((name, sum(1 for _ in nc.all_instructions())))
    except Exception:
        pass


def _body(nc, tc, d):
    import contextlib
    with contextlib.ExitStack() as ctx:
        Pp = ctx.enter_context(tc.tile_pool(name="persist", bufs=1))

        t_ob = Pp.tile([128, TK], F32, tag="obias")
        t_dbg = Pp.tile([128, HK], F32, tag="dbg")
        t_emb = Pp.tile([128, HK, R], F8, tag="emb")
        t_h = [Pp.tile([128, HK, R], F8, tag=f"hT{i}", name=f"hT{i}")
               for i in (0, 1)]
        t_c = Pp.tile([128, HK, R], F32, tag="c")
        t_note = Pp.tile([128, TK, R], BF, tag="note")
        t_note8 = Pp.tile([128, TK, R], F8, tag="note8")
        # ge persists through the decoder; filled per-level in the conductor
        t_ge = Pp.tile([128, 3 * HK, R], BF, tag="ge")
        # decoder weights needed at dec00 (loaded during the conductor)
        t_dwhh = Pp.tile([128, HK, 4 * H], F8, tag="dwhh")
        t_dweg = Pp.tile([128, HK, H], F8, tag="dweg")

        # ---------------- conductor (+ per-level ge fill) ----------------
        with tc.tile_pool(name="cond", bufs=1) as Pc, \
             tc.tile_pool(name="ctmp", bufs=2) as Pt:
            # DMA order on the shared engine: gz deps first, then cwhh
            # (needed at level 1), then dwe (ge fills), then the rest.
            t_cwhh = Pc.tile([128, HK, 4 * H], F8, tag="cwhh")
            t_ones = Pc.tile([1, Bc], BF, tag="ones")
            t_gebb = Pc.tile([1, 3 * H], BF, tag="gebb")
            # gz chunks indexed [p, gate] with gate order (i, f, o, g)
            t_gz = Pc.tile([128, HK, 4, R], BF, tag="gz")
            t_cc = Pc.tile([128, HK, Bc], F32, tag="cc")

            # gz = z @ cond_Wih.T + cond_b for all levels at once (fp8 DR);
            # bias folded via Identity-activation copy from PSUM.
            _mark(nc, "gz")
            with tc.tile_pool(name="condz", bufs=1) as Pcz, \
                 tc.tile_pool(name="gzps", bufs=2, space="PSUM") as PSz:
                t_cwih = Pcz.tile([128, ZK, 4 * H], F8, tag="cwih")
                t_zT = Pcz.tile([128, ZK, R], F8, tag="zT")
                nc.sync.dma_start(t_zT[:], d["zT"][:])
                t_cbt = Pcz.tile([128, G], F32, tag="cbt")
                nc.sync.dma_start(t_cbt[:], d["cbt"][:])
                # cwih in halves so the first gz chunks start ~4us earlier
                half = 2 * H
                nc.sync.dma_start(t_cwih[:, :, 0:half],
                                  d["cwih"][:, :, 0:half])
                nc.sync.dma_start(t_cwih[:, :, half:4 * H],
                                  d["cwih"][:, :, half:4 * H])
                nc.sync.dma_start(t_cwhh[:], d["cwhh"][:])
                nc.sync.dma_start(t_ones[:], d["ones"][:])
                nc.sync.dma_start(t_gebb[:], d["gebb"][:])
                for m in range(G):
                    ms = slice(m * 128, (m + 1) * 128)
                    ps = PSz.tile([128, R], F32, tag="gzp", name="gzp")
                    for k in range(0, ZK, 2):
                        _mm_dr(nc, ps[:], t_cwih, t_zT, k, ms,
                               (k == 0), (k == ZK - 2))
                    nc.scalar.activation(t_gz[:, m // 4, m % 4, :], ps[:],
                                         AF.Identity, bias=t_cbt[:, m:m + 1])

            # sequential levels (fp8 non-DR: 32-row streams), elementwise
            # batched level-wide via strided views; each level's ge slice is
            # computed on the PE right after its emb is ready, filling the
            # PE idle while the next level's elementwise chain runs.
            _mark(nc, "conductor")
            with tc.tile_pool(name="dwepool", bufs=1) as Pdwe, \
                 tc.tile_pool(name="cps", bufs=3, space="PSUM") as PSc, \
                 tc.tile_pool(name="geps", bufs=1, space="PSUM") as PSg:
                # dwe lives only through the conductor levels (ge fills)
                t_dwe = Pdwe.tile([128, HK, 3 * H], F8, tag="dwe")
                nc.sync.dma_start(t_dwe[:], d["dwe"][:])
                # decoder weights/state needed at dec00 load during the levels
                nc.sync.dma_start(t_h[0][:], d["h0T"][:])
                nc.sync.dma_start(t_c[:], d["c0T"][:])
                nc.sync.dma_start(t_ob[:], d["obias"][:])
                nc.sync.dma_start(t_dbg[:], d["dbg"][:])
                nc.sync.dma_start(t_dwhh[:], d["dwhh"][:])
                nc.sync.dma_start(t_dweg[:], d["dweg"][:])

                def ge_fill(lv):
                    cs = slice(lv * Bc, (lv + 1) * Bc)
                    gp = PSg.tile([128, 3 * HK, Bc], F32, tag="gep",
                                  name="gep")
                    for m in range(3 * HK):
                        nc.tensor.matmul(gp[:, m, :],
                                         t_gebb[0:1, m * 128:(m + 1) * 128],
                                         t_ones[:], start=True, stop=False)
                        for k in range(0, HK, 2):
                            nc.tensor.matmul(
                                gp[:, m, :],
                                t_dwe[:, k:k + 2, m * 128:(m + 1) * 128],
                                t_emb[:, k:k + 2, cs],
                                start=False, stop=(k == HK - 2), perf_mode=DR)
                    nc.vector.tensor_copy(t_ge[:, :, cs], gp[:])

                for _crep in range(COND_REPS):
                  for lv in range(L):
                      cs = slice(lv * Bc, (lv + 1) * Bc)
                      ps_prev = slice((lv - 1) * Bc, lv * Bc)
                      tsig = Pt.tile([128, HK, 3, Bc], BF, tag="tsig",
                                     name="tsig")
                      tg = Pt.tile([128, HK, Bc], BF, tag="tg", name="tg")
                      tcn = Pt.tile([128, HK, Bc], BF, tag="tcn", name="tcn")
                      if lv == 0:
                          # h0 == 0: gates are just gz; c0 == 0
                          nc.scalar.activation(tsig[:], t_gz[:, :, 0:3, cs],
                                               AF.Sigmoid)
                          nc.scalar.activation(tg[:], t_gz[:, :, 3, cs],
                                               AF.Tanh)
                          nc.vector.tensor_mul(t_cc[:], tsig[:, :, 0, :],
                                               tg[:])
                          nc.scalar.activation(tcn[:], t_cc[:], AF.Tanh)
                          nc.vector.tensor_mul(t_emb[:, :, cs],
                                               tsig[:, :, 2, :], tcn[:])
                      else:
                          ps = PSc.tile([128, HK, 4, Bc], F32, tag="cgp",
                                        name="cgp")
                          for p in range(HK):
                              for g in range(4):
                                  ms = slice((p * 4 + g) * 128,
                                             (p * 4 + g + 1) * 128)
                                  for k in range(0, HK, 2):
                                      nc.tensor.matmul(
                                          ps[:, p, g, :],
                                          t_cwhh[:, k:k + 2, ms],
                                          t_emb[:, k:k + 2, ps_prev],
                                          start=(k == 0),
                                          stop=(k == HK - 2), perf_mode=DR)
                          # ge fill lagged two levels: keeps it off the
                          # dwe-DMA critical window at conductor start
                          if lv >= 2:
                              ge_fill(lv - 2)
                          gs = Pt.tile([128, HK, 4, Bc], BF, tag="gs",
                                       name="gs")
                          tm1 = Pt.tile([128, HK, Bc], BF, tag="tm1",
                                        name="tm1")
                          tm2 = Pt.tile([128, HK, Bc], F32, tag="tm2",
                                        name="tm2")
                          # elementwise in two p-halves, pipelined across
                          # engines; emb half 0 lands early so the next
                          # level's k-outer matmuls can begin
                          for hp in (slice(0, HK // 2), slice(HK // 2, HK)):
                              nc.vector.tensor_add(gs[:, hp, :, :],
                                                   ps[:, hp, :, :],
                                                   t_gz[:, hp, :, cs])
                              nc.scalar.activation(tsig[:, hp, :, :],
                                                   gs[:, hp, 0:3, :],
                                                   AF.Sigmoid)
                              nc.scalar.activation(tg[:, hp, :],
                                                   gs[:, hp, 3, :], AF.Tanh)
                              nc.vector.tensor_mul(tm1[:, hp, :],
                                                   tsig[:, hp, 0, :],
                                                   tg[:, hp, :])
                              nc.gpsimd.tensor_mul(tm2[:, hp, :],
                                                   tsig[:, hp, 1, :],
                                                   t_cc[:, hp, :])
                              nc.vector.tensor_add(t_cc[:, hp, :],
                                                   tm1[:, hp, :],
                                                   tm2[:, hp, :])
                              nc.scalar.activation(tcn[:, hp, :],
                                                   t_cc[:, hp, :], AF.Tanh)
                              nc.vector.tensor_mul(t_emb[:, hp, cs],
                                                   tsig[:, hp, 2, :],
                                                   tcn[:, hp, :])
                  ge_fill(L - 2)
                  ge_fill(L - 1)

        # remaining decoder weights (first used at dec00 outproj / dec01)
        Pw2 = ctx.enter_context(tc.tile_pool(name="wdec2", bufs=1))
        t_dwn = Pw2.tile([128, TK, 4 * H], F8, tag="dwn")
        nc.sync.dma_start(t_dwn[:], d["dwn"][:])
        t_owt = Pw2.tile([128, HK, T], F8, tag="owt")
        nc.sync.dma_start(t_owt[:], d["owt"][:])

        # ---------------- decoder: 16 note steps over 512 rows --------------
        with tc.tile_pool(name="dtmp", bufs=4) as Pdt, \
             tc.tile_pool(name="dps", bufs=3, space="PSUM") as PSd, \
             tc.tile_pool(name="dpso", bufs=2, space="PSUM") as PSo:
            prefetched = {}
            for _drep in range(DEC_REPS):
              for t in range(NS):
                  _mark(nc, f"dec{t:02d}")
                  hin = t_h[t % 2]
                  hout = t_h[(t + 1) % 2]
                  psAB = {}
                  # software-pipelined emission: stage ops of chunk p are
                  # emitted after stage ops of chunk p+1's predecessors so
                  # each engine's FIFO never head-of-line blocks on a
                  # dependency that a later-emitted independent op could fill.
                  tiles = {}

                  def mms(p):
                      psA = PSd.tile([128, 2, R], F32, tag="dgp", name="psA")
                      if p in prefetched:
                          psB = prefetched.pop(p)
                      else:
                          psB = PSd.tile([128, 2, R], F32, tag="dgp",
                                         name="psB")
                          # g gate: emb contribution recomputed on PE
                          for k in range(0, HK, 2):
                              _mm_dr(nc, psB[:, 1, :], t_dweg, t_emb,
                                     k, slice(p * 128, (p + 1) * 128),
                                     (k == 0), False)
                      psAB[p] = (psA, psB)
                      for gi in range(4):
                          pst = psA if gi < 2 else psB
                          sl = gi % 2
                          ms = slice((p * 4 + gi) * 128,
                                     (p * 4 + gi + 1) * 128)
                          for k in range(0, HK, 2):
                              _mm_dr(nc, pst[:, sl, :], t_dwhh, hin, k, ms,
                                     (k == 0 and gi != 3),
                                     (t == 0 and k == HK - 2))
                          if t > 0:
                              for k in range(0, TK, 2):
                                  _mm_dr(nc, pst[:, sl, :], t_dwn, t_note8,
                                         k, ms, False, (k == TK - 2))

                  FULL = slice(0, R)
                  HALVES = (slice(0, R // 2), slice(R // 2, R))

                  def adds(p, cl=FULL, alloc=True):
                      psA, psB = psAB[p]
                      if alloc:
                          gs3 = Pdt.tile([128, 3, R], BF, tag="gs3",
                                         name="gs3")
                          tg = Pdt.tile([128, R], BF, tag="tg", name="tg")
                          tiles[p] = (gs3, tg)
                      gs3, tg = tiles[p]
                      nc.vector.tensor_add(gs3[:, 2, cl], psB[:, 0, cl],
                                           t_ge[:, 3 * p + 2, cl])
                      nc.vector.tensor_add(gs3[:, 0:2, cl], psA[:, :, cl],
                                           t_ge[:, 3 * p:3 * p + 2, cl])
                      nc.scalar.activation(tg[:, cl], psB[:, 1, cl], AF.Tanh,
                                           bias=t_dbg[:, p:p + 1])

                  def acts(p, cl=FULL, alloc=True):
                      if alloc:
                          gs3, tg = tiles[p]
                          tsig = Pdt.tile([128, 3, R], BF, tag="tsig3",
                                          name="tsig3")
                          tiles[p] = (tsig, tg, gs3)
                      tsig, tg, gs3 = tiles[p]
                      nc.scalar.activation(tsig[:, :, cl], gs3[:, :, cl],
                                           AF.Sigmoid)

                  def tail(p, cl=FULL, alloc=True):
                      tsig, tg = tiles[p][0], tiles[p][1]
                      if alloc:
                          tcn = Pdt.tile([128, R], BF, tag="tcn", name="tcn")
                          tm1 = Pdt.tile([128, R], BF, tag="tm1", name="tm1")
                          tm2 = Pdt.tile([128, R], F32, tag="tm2", name="tm2")
                          tiles[(p, 'x')] = (tcn, tm1, tm2)
                      tcn, tm1, tm2 = tiles[(p, 'x')]
                      nc.vector.tensor_mul(tm1[:, cl], tsig[:, 0, cl],
                                           tg[:, cl])
                      nc.gpsimd.tensor_mul(tm2[:, cl], tsig[:, 1, cl],
                                           t_c[:, p, cl])
                      ceng = nc.vector if p >= HK - 2 else nc.gpsimd
                      ceng.tensor_add(t_c[:, p, cl], tm1[:, cl],
                                      tm2[:, cl])
                      nc.scalar.activation(tcn[:, cl], t_c[:, p, cl], AF.Tanh)
                      nc.vector.tensor_mul(hout[:, p, cl], tsig[:, 2, cl],
                                           tcn[:, cl])

                  for p in range(HK + 3):
                      if p < HK:
                          mms(p)
                          if p < HK - 2:
                              adds(p)
                          else:
                              # last two chunks: half-R ops so the
                              # step-boundary chain pipelines at finer grain
                              adds(p, HALVES[0])
                              adds(p, HALVES[1], alloc=False)
                      if 1 <= p and p - 1 < HK:
                          if p - 1 < HK - 2:
                              acts(p - 1)
                          else:
                              acts(p - 1, HALVES[0])
                              acts(p - 1, HALVES[1], alloc=False)
                      if p >= 3:
                          q = p - 3
                          if q < HK - 2:
                              tail(q)
                          else:
                              tail(q, HALVES[0])
                              tail(q, HALVES[1], alloc=False)
                  # output projection + sigmoid -> note (fp8 mirror first so
                  # the next step's Wn matmuls unblock as early as possible)
                  for tk in range(TK):
                      ts_ = slice(tk * 128, (tk + 1) * 128)
                      po = PSo.tile([128, R], F32, tag="dpo", name="dpo")
                      for k in range(0, HK, 2):
                          _mm_dr(nc, po[:], t_owt, hout, k, ts_,
                                 (k == 0), (k == HK - 2))
                      nc.scalar.activation(t_note[:, tk, :], po[:],
                                           AF.Sigmoid, bias=t_ob[:, tk:tk + 1])
                      if t + 1 < NS:
                          nc.vector.tensor_copy(t_note8[:, tk, :],
                                                t_note[:, tk, :])
                      nc.sync.dma_start(d["outbuf"][t, tk], t_note[:, tk, :])


import os
DEC_REPS = int(os.environ.get("KBENCH_DEC_REPS", "1"))
COND_REPS = int(os.environ.get("KBENCH_COND_REPS", "1"))

_CACHE = {}


def _build():
    if "nc" not in _CACHE:
        nc = bacc.Bacc("TRN2", target_bir_lowering=False, debug=False,
                       num_devices=NCORES)
        d = _declare(nc)
        with tile.TileContext(nc) as tc:
            _body(nc, tc, d)
        nc.compile()
        _CACHE["nc"] = nc
    return _CACHE["nc"]


def _feat_major(W, dt):
    """[J, K] -> [128, K/128, J] (stationary lhsT chunk layout)."""
    J, K = W.shape
    return np.ascontiguousarray(
        W.reshape(J, K // 128, 128).transpose(2, 1, 0)).astype(dt)


def _reorder4(W):
    """[4H, K] rows in PyTorch gate blocks (i,f,g,o) -> p-adjacent chunks in
    order (i,f,o,g): new chunk m = p*4 + {0:i,1:f,2:o,3:g}."""
    K = W.shape[1]
    W4 = W.reshape(4, HK, 128, K)[[0, 1, 3, 2]]
    return np.ascontiguousarray(W4.transpose(1, 0, 2, 3).reshape(4 * H, K))


def _pack_inputs(inputs):
    z = np.asarray(inputs["z"], np.float32)
    dec_h0 = np.asarray(inputs["dec_h0"], np.float32)
    dec_c0 = np.asarray(inputs["dec_c0"], np.float32)
    cond_b = np.asarray(inputs["cond_bih"] + inputs["cond_bhh"], np.float32)
    dec_b = np.asarray(inputs["dec_bih"] + inputs["dec_bhh"], np.float32)
    out_b = np.asarray(inputs["out_b"], np.float32)

    cb4 = cond_b.reshape(4, HK, 128)[[0, 1, 3, 2]]
    cbt = np.ascontiguousarray(cb4.transpose(1, 0, 2).reshape(G, 128).T)
    db4 = dec_b.reshape(4, HK, 128)
    gebb = np.ascontiguousarray(
        db4[[0, 1, 3]].transpose(1, 0, 2).reshape(1, 3 * H))
    dbg = np.ascontiguousarray(db4[2].T)

    We = np.asarray(inputs["dec_Wih"][:, :H], np.float32)
    We4 = We.reshape(4, HK, 128, H)
    We3 = np.ascontiguousarray(
        We4[[0, 1, 3]].transpose(1, 0, 2, 3).reshape(3 * H, H))
    dwe = _feat_major(We3, f8)                      # [128, HK, 3H]
    dweg = _feat_major(np.ascontiguousarray(We4[2].reshape(H, H)), f8)

    shared = {
        "cbt": cbt.astype(np.float32),
        "dbg": dbg.astype(np.float32),
        "ones": np.ones((1, Bc), dtype=bf16),
        "gebb": gebb.astype(bf16),
        "obias": np.ascontiguousarray(out_b.reshape(TK, 128).T).astype(np.float32),
        "cwih": _feat_major(_reorder4(np.asarray(inputs["cond_Wih"], np.float32)), f8),
        "cwhh": _feat_major(_reorder4(np.asarray(inputs["cond_Whh"], np.float32)), f8),
        "dwn": _feat_major(_reorder4(np.asarray(inputs["dec_Wih"][:, H:], np.float32)), f8),
        "dwhh": _feat_major(_reorder4(np.asarray(inputs["dec_Whh"], np.float32)), f8),
        "owt": _feat_major(np.asarray(inputs["out_W"], np.float32), f8),
        "dwe": dwe,
        "dweg": dweg,
    }

    z_lv = z[:, np.arange(L) * L, 0, :]           # [B, L, Z]
    in_maps = []
    for c in range(NCORES):
        bs = slice(c * Bc, (c + 1) * Bc)
        zc = z_lv[bs]                              # [Bc, L, Z]
        zT = np.ascontiguousarray(
            zc.reshape(Bc, L, ZK, 128).transpose(3, 2, 1, 0).reshape(128, ZK, R)
        ).astype(f8)
        h0 = dec_h0[:, bs, :]                      # [L, Bc, H]
        h0T = np.ascontiguousarray(
            h0.reshape(L, Bc, HK, 128).transpose(3, 2, 0, 1).reshape(128, HK, R))
        c0 = dec_c0[:, bs, :]
        c0T = np.ascontiguousarray(
            c0.reshape(L, Bc, HK, 128).transpose(3, 2, 0, 1).reshape(128, HK, R))
        m = dict(shared)
        m["zT"] = zT
        m["h0T"] = h0T.astype(f8)
        m["c0T"] = c0T.astype(np.float32)
        in_maps.append(m)
    return in_maps


def _unpack_outputs(core_outs):
    notes = np.empty((B, L * NS, T), np.float32)
    for c, arr in enumerate(core_outs):
        # arr [NS, TK, 128, R] -> [Bc, L, NS, T]
        a = arr.astype(np.float32).reshape(NS, TK, 128, L, Bc).transpose(4, 3, 0, 1, 2)
        notes[c * Bc:(c + 1) * Bc] = a.reshape(Bc, L, NS, T).reshape(
            Bc, L * NS, T)
    return notes


def kernel(**inputs):
    nc = _build()
    in_maps = _pack_inputs(inputs)
    res = run_bass_kernel_spmd(nc, in_maps, list(range(NCORES)))
    return _unpack_outputs([r["outbuf"] for r in res.results])


# revision 57
# speedup vs baseline: 1.0571x; 1.0092x over previous
"""Trainium2 Bass kernel for nn_Decoder (MusicVAE-style hierarchical decoder).

Strategy (8 NeuronCores, data-parallel over batch, no inter-core comms):
  - Conductor LSTM (16 sequential levels, batch 32/core) computes per-level
    embeddings; decoder levels are independent, so all 16 levels are batched:
    512 decoder rows per core, 16 sequential note steps.
  - fp8(e4m3) matmuls, DoubleRow perf mode for 512-row streams; fp32 PSUM.
  - Gate chunks are laid out p-adjacent in order (i, f, o, g) so the three
    sigmoid gates batch into one activation op and (i,f) / (o,g) pairs map
    onto two 2-bank PSUM accumulation tiles.
  - The g-gate's conductor-embedding contribution (emb @ dec_Wih_g.T) is
    recomputed on the PE every step (cheaper than a DVE add at model rates);
    its bias rides the tanh activation's bias port.  i/f/o biases are folded
    into ge / gz via Identity-activation copies (no bias matmuls, no ones).
  - Elementwise work is spread across DVE (vector), Pool (gpsimd) and
    Activation engines to balance the per-step makespan against the PE.
"""
import numpy as np
import ml_dtypes

import concourse.bacc as bacc
import concourse.tile as tile
import concourse.mybir as mybir
from concourse.bass_utils import run_bass_kernel_spmd

bf16 = ml_dtypes.bfloat16
f8 = ml_dtypes.float8_e4m3
F32 = mybir.dt.float32
BF = mybir.dt.bfloat16
F8 = mybir.dt.float8e4
AF = mybir.ActivationFunctionType
DR = mybir.MatmulPerfMode.DoubleRow

NCORES = 8
B, Z, H, T = 256, 512, 1024, 512
L, NS = 16, 16
Bc = B // NCORES            # 32 batch rows per core
R = L * Bc                  # 512 decoder rows per core (levels x batch)
HK, TK, ZK = H // 128, T // 128, Z // 128   # 8, 4, 4
G = 4 * H // 128            # 32 gate chunks of 128


def _declare(nc):
    d = {}
    ei = dict(kind="ExternalInput")
    d["cbt"] = nc.dram_tensor("cbt", [128, G], F32, **ei)
    d["dbg"] = nc.dram_tensor("dbg", [128, HK], F32, **ei)
    d["obias"] = nc.dram_tensor("obias", [128, TK], F32, **ei)
    d["zT"] = nc.dram_tensor("zT", [128, ZK, R], F8, **ei)
    d["h0T"] = nc.dram_tensor("h0T", [128, HK, R], F8, **ei)
    d["c0T"] = nc.dram_tensor("c0T", [128, HK, R], F32, **ei)
    d["cwih"] = nc.dram_tensor("cwih", [128, ZK, 4 * H], F8, **ei)
    d["cwhh"] = nc.dram_tensor("cwhh", [128, HK, 4 * H], F8, **ei)
    d["ones"] = nc.dram_tensor("ones", [1, Bc], BF, **ei)
    d["gebb"] = nc.dram_tensor("gebb", [1, 3 * H], BF, **ei)
    d["dwe"] = nc.dram_tensor("dwe", [128, HK, 3 * H], F8, **ei)
    d["dweg"] = nc.dram_tensor("dweg", [128, HK, H], F8, **ei)
    d["dwn"] = nc.dram_tensor("dwn", [128, TK, 4 * H], F8, **ei)
    d["dwhh"] = nc.dram_tensor("dwhh", [128, HK, 4 * H], F8, **ei)
    d["owt"] = nc.dram_tensor("owt", [128, HK, T], F8, **ei)
    d["outbuf"] = nc.dram_tensor("outbuf", [NS, TK, 128, R], BF,
                                 kind="ExternalOutput")
    return d


def _mm_dr(nc, out, w3, x3, ks, ms, start, stop):
    """DoubleRow fp8 matmul over k-subtile pair (ks, ks+1)."""
    return nc.tensor.matmul(out, w3[:, ks:ks + 2, ms], x3[:, ks:ks + 2, :],
                            start=start, stop=stop, perf_mode=DR)


PHASE_MARKS = []


def _mark(nc, name):
    try:
        PHASE_MARKS.append((name, sum(1 for _ in nc.all_instructions())))
    except Exception:
        pass


def _body(nc, tc, d):
    import contextlib
    with contextlib.ExitStack() as ctx:
        Pp = ctx.enter_context(tc.tile_pool(name="persist", bufs=1))

        t_ob = Pp.tile([128, TK], F32, tag="obias")
        t_dbg = Pp.tile([128, HK], F32, tag="dbg")
        t_emb = Pp.tile([128, HK, R], F8, tag="emb")
        t_h = [Pp.tile([128, HK, R], F8, tag=f"hT{i}", name=f"hT{i}")
               for i in (0, 1)]
        t_c = Pp.tile([128, HK, R], F32, tag="c")
        t_note = Pp.tile([128, TK, R], BF, tag="note")
        t_note8 = Pp.tile([128, TK, R], F8, tag="note8")
        # ge persists through the decoder; filled per-level in the conductor
        t_ge = Pp.tile([128, 3 * HK, R], BF, tag="ge")
        # decoder weights needed at dec00 (loaded during the conductor)
        t_dwhh = Pp.tile([128, HK, 4 * H], F8, tag="dwhh")
        t_dweg = Pp.tile([128, HK, H], F8, tag="dweg")

        # ---------------- conductor (+ per-level ge fill) ----------------
        with tc.tile_pool(name="cond", bufs=1) as Pc, \
             tc.tile_pool(name="ctmp", bufs=2) as Pt:
            # DMA order on the shared engine: gz deps first, then cwhh
            # (needed at level 1), then dwe (ge fills), then the rest.
            t_cwhh = Pc.tile([128, HK, 4 * H], F8, tag="cwhh")
            t_ones = Pc.tile([1, Bc], BF, tag="ones")
            t_gebb = Pc.tile([1, 3 * H], BF, tag="gebb")
            # gz chunks indexed [p, gate] with gate order (i, f, o, g)
            t_gz = Pc.tile([128, HK, 4, R], BF, tag="gz")
            t_cc = Pc.tile([128, HK, Bc], F32, tag="cc")

            # gz = z @ cond_Wih.T + cond_b for all levels at once (fp8 DR);
            # bias folded via Identity-activation copy from PSUM.
            _mark(nc, "gz")
            with tc.tile_pool(name="condz", bufs=1) as Pcz, \
                 tc.tile_pool(name="gzps", bufs=2, space="PSUM") as PSz:
                t_cwih = Pcz.tile([128, ZK, 4 * H], F8, tag="cwih")
                t_zT = Pcz.tile([128, ZK, R], F8, tag="zT")
                nc.sync.dma_start(t_zT[:], d["zT"][:])
                t_cbt = Pcz.tile([128, G], F32, tag="cbt")
                nc.sync.dma_start(t_cbt[:], d["cbt"][:])
                # cwih in halves so the first gz chunks start ~4us earlier
                half = 2 * H
                nc.sync.dma_start(t_cwih[:, :, 0:half],
                                  d["cwih"][:, :, 0:half])
                nc.sync.dma_start(t_cwih[:, :, half:4 * H],
                                  d["cwih"][:, :, half:4 * H])
                nc.sync.dma_start(t_cwhh[:], d["cwhh"][:])
                nc.sync.dma_start(t_ones[:], d["ones"][:])
                nc.sync.dma_start(t_gebb[:], d["gebb"][:])
                for m in range(G):
                    ms = slice(m * 128, (m + 1) * 128)
                    ps = PSz.tile([128, R], F32, tag="gzp", name="gzp")
                    for k in range(0, ZK, 2):
                        _mm_dr(nc, ps[:], t_cwih, t_zT, k, ms,
                               (k == 0), (k == ZK - 2))
                    nc.scalar.activation(t_gz[:, m // 4, m % 4, :], ps[:],
                                         AF.Identity, bias=t_cbt[:, m:m + 1])

            # sequential levels (fp8 non-DR: 32-row streams), elementwise
            # batched level-wide via strided views; each level's ge slice is
            # computed on the PE right after its emb is ready, filling the
            # PE idle while the next level's elementwise chain runs.
            _mark(nc, "conductor")
            with tc.tile_pool(name="dwepool", bufs=1) as Pdwe, \
                 tc.tile_pool(name="cps", bufs=3, space="PSUM") as PSc, \
                 tc.tile_pool(name="geps", bufs=1, space="PSUM") as PSg:
                # dwe lives only through the conductor levels (ge fills)
                t_dwe = Pdwe.tile([128, HK, 3 * H], F8, tag="dwe")
                nc.sync.dma_start(t_dwe[:], d["dwe"][:])
                # decoder weights/state needed at dec00 load during the levels
                nc.sync.dma_start(t_h[0][:], d["h0T"][:])
                nc.sync.dma_start(t_c[:], d["c0T"][:])
                nc.sync.dma_start(t_ob[:], d["obias"][:])
                nc.sync.dma_start(t_dbg[:], d["dbg"][:])
                nc.sync.dma_start(t_dwhh[:], d["dwhh"][:])
                nc.sync.dma_start(t_dweg[:], d["dweg"][:])

                def ge_fill(lv):
                    cs = slice(lv * Bc, (lv + 1) * Bc)
                    gp = PSg.tile([128, 3 * HK, Bc], F32, tag="gep",
                                  name="gep")
                    for m in range(3 * HK):
                        nc.tensor.matmul(gp[:, m, :],
                                         t_gebb[0:1, m * 128:(m + 1) * 128],
                                         t_ones[:], start=True, stop=False)
                        for k in range(0, HK, 2):
                            nc.tensor.matmul(
                                gp[:, m, :],
                                t_dwe[:, k:k + 2, m * 128:(m + 1) * 128],
                                t_emb[:, k:k + 2, cs],
                                start=False, stop=(k == HK - 2), perf_mode=DR)
                    nc.vector.tensor_copy(t_ge[:, :, cs], gp[:])

                for _crep in range(COND_REPS):
                  for lv in range(L):
                      cs = slice(lv * Bc, (lv + 1) * Bc)
                      ps_prev = slice((lv - 1) * Bc, lv * Bc)
                      tsig = Pt.tile([128, HK, 3, Bc], BF, tag="tsig",
                                     name="tsig")
                      tg = Pt.tile([128, HK, Bc], BF, tag="tg", name="tg")
                      tcn = Pt.tile([128, HK, Bc], BF, tag="tcn", name="tcn")
                      if lv == 0:
                          # h0 == 0: gates are just gz; c0 == 0
                          nc.scalar.activation(tsig[:], t_gz[:, :, 0:3, cs],
                                               AF.Sigmoid)
                          nc.scalar.activation(tg[:], t_gz[:, :, 3, cs],
                                               AF.Tanh)
                          nc.vector.tensor_mul(t_cc[:], tsig[:, :, 0, :],
                                               tg[:])
                          nc.scalar.activation(tcn[:], t_cc[:], AF.Tanh)
                          nc.vector.tensor_mul(t_emb[:, :, cs],
                                               tsig[:, :, 2, :], tcn[:])
                      else:
                          ps = PSc.tile([128, HK, 4, Bc], F32, tag="cgp",
                                        name="cgp")
                          for p in range(HK):
                              for g in range(4):
                                  ms = slice((p * 4 + g) * 128,
                                             (p * 4 + g + 1) * 128)
                                  for k in range(0, HK, 2):
                                      nc.tensor.matmul(
                                          ps[:, p, g, :],
                                          t_cwhh[:, k:k + 2, ms],
                                          t_emb[:, k:k + 2, ps_prev],
                                          start=(k == 0),
                                          stop=(k == HK - 2), perf_mode=DR)
                          # ge fill lagged two levels: keeps it off the
                          # dwe-DMA critical window at conductor start
                          if lv >= 2:
                              ge_fill(lv - 2)
                          gs = Pt.tile([128, HK, 4, Bc], BF, tag="gs",
                                       name="gs")
                          tm1 = Pt.tile([128, HK, Bc], BF, tag="tm1",
                                        name="tm1")
                          tm2 = Pt.tile([128, HK, Bc], F32, tag="tm2",
                                        name="tm2")
                          # elementwise in two p-halves, pipelined across
                          # engines; emb half 0 lands early so the next
                          # level's k-outer matmuls can begin
                          for hp in (slice(0, HK // 2), slice(HK // 2, HK)):
                              nc.vector.tensor_add(gs[:, hp, :, :],
                                                   ps[:, hp, :, :],
                                                   t_gz[:, hp, :, cs])
                              nc.scalar.activation(tsig[:, hp, :, :],
                                                   gs[:, hp, 0:3, :],
                                                   AF.Sigmoid)
                              nc.scalar.activation(tg[:, hp, :],
                                                   gs[:, hp, 3, :], AF.Tanh)
                              nc.vector.tensor_mul(tm1[:, hp, :],
                                                   tsig[:, hp, 0, :],
                                                   tg[:, hp, :])
                              nc.gpsimd.tensor_mul(tm2[:, hp, :],
                                                   tsig[:, hp, 1, :],
                                                   t_cc[:, hp, :])
                              nc.vector.tensor_add(t_cc[:, hp, :],
                                                   tm1[:, hp, :],
                                                   tm2[:, hp, :])
                              nc.scalar.activation(tcn[:, hp, :],
                                                   t_cc[:, hp, :], AF.Tanh)
                              nc.vector.tensor_mul(t_emb[:, hp, cs],
                                                   tsig[:, hp, 2, :],
                                                   tcn[:, hp, :])
                  ge_fill(L - 2)
                  ge_fill(L - 1)

        # remaining decoder weights (first used at dec00 outproj / dec01)
        Pw2 = ctx.enter_context(tc.tile_pool(name="wdec2", bufs=1))
        t_dwn = Pw2.tile([128, TK, 4 * H], F8, tag="dwn")
        nc.sync.dma_start(t_dwn[:], d["dwn"][:])
        t_owt = Pw2.tile([128, HK, T], F8, tag="owt")
        nc.sync.dma_start(t_owt[:], d["owt"][:])

        # ---------------- decoder: 16 note steps over 512 rows --------------
        with tc.tile_pool(name="dtmp", bufs=4) as Pdt, \
             tc.tile_pool(name="dps", bufs=3, space="PSUM") as PSd, \
             tc.tile_pool(name="dpso", bufs=2, space="PSUM") as PSo:
            prefetched = {}
            for _drep in range(DEC_REPS):
              for t in range(NS):
                  _mark(nc, f"dec{t:02d}")
                  hin = t_h[t % 2]
                  hout = t_h[(t + 1) % 2]
                  psAB = {}
                  # software-pipelined emission: stage ops of chunk p are
                  # emitted after stage ops of chunk p+1's predecessors so
                  # each engine's FIFO never head-of-line blocks on a
                  # dependency that a later-emitted independent op could fill.
                  tiles = {}

                  def mms(p):
                      psA = PSd.tile([128, 2, R], F32, tag="dgp", name="psA")
                      if p in prefetched:
                          psB = prefetched.pop(p)
                      else:
                          psB = PSd.tile([128, 2, R], F32, tag="dgp",
                                         name="psB")
                          # g gate: emb contribution recomputed on PE
                          for k in range(0, HK, 2):
                              _mm_dr(nc, psB[:, 1, :], t_dweg, t_emb,
                                     k, slice(p * 128, (p + 1) * 128),
                                     (k == 0), False)
                      psAB[p] = (psA, psB)
                      for gi in range(4):
                          pst = psA if gi < 2 else psB
                          sl = gi % 2
                          ms = slice((p * 4 + gi) * 128,
                                     (p * 4 + gi + 1) * 128)
                          for k in range(0, HK, 2):
                              _mm_dr(nc, pst[:, sl, :], t_dwhh, hin, k, ms,
                                     (k == 0 and gi != 3),
                                     (t == 0 and k == HK - 2))
                          if t > 0:
                              for k in range(0, TK, 2):
                                  _mm_dr(nc, pst[:, sl, :], t_dwn, t_note8,
                                         k, ms, False, (k == TK - 2))

                  FULL = slice(0, R)
                  HALVES = (slice(0, 3 * R // 4), slice(3 * R // 4, R))

                  def adds(p, cl=FULL, alloc=True):
                      psA, psB = psAB[p]
                      if alloc:
                          gs3 = Pdt.tile([128, 3, R], BF, tag="gs3",
                                         name="gs3")
                          tg = Pdt.tile([128, R], BF, tag="tg", name="tg")
                          tiles[p] = (gs3, tg)
                      gs3, tg = tiles[p]
                      nc.vector.tensor_add(gs3[:, 2, cl], psB[:, 0, cl],
                                           t_ge[:, 3 * p + 2, cl])
                      nc.vector.tensor_add(gs3[:, 0:2, cl], psA[:, :, cl],
                                           t_ge[:, 3 * p:3 * p + 2, cl])
                      nc.scalar.activation(tg[:, cl], psB[:, 1, cl], AF.Tanh,
                                           bias=t_dbg[:, p:p + 1])

                  def acts(p, cl=FULL, alloc=True):
                      if alloc:
                          gs3, tg = tiles[p]
                          tsig = Pdt.tile([128, 3, R], BF, tag="tsig3",
                                          name="tsig3")
                          tiles[p] = (tsig, tg, gs3)
                      tsig, tg, gs3 = tiles[p]
                      nc.scalar.activation(tsig[:, :, cl], gs3[:, :, cl],
                                           AF.Sigmoid)

                  def tail(p, cl=FULL, alloc=True):
                      tsig, tg = tiles[p][0], tiles[p][1]
                      if alloc:
                          tcn = Pdt.tile([128, R], BF, tag="tcn", name="tcn")
                          tm1 = Pdt.tile([128, R], BF, tag="tm1", name="tm1")
                          tm2 = Pdt.tile([128, R], F32, tag="tm2", name="tm2")
                          tiles[(p, 'x')] = (tcn, tm1, tm2)
                      tcn, tm1, tm2 = tiles[(p, 'x')]
                      nc.vector.tensor_mul(tm1[:, cl], tsig[:, 0, cl],
                                           tg[:, cl])
                      nc.gpsimd.tensor_mul(tm2[:, cl], tsig[:, 1, cl],
                                           t_c[:, p, cl])
                      ceng = nc.vector if p >= HK - 2 else nc.gpsimd
                      ceng.tensor_add(t_c[:, p, cl], tm1[:, cl],
                                      tm2[:, cl])
                      nc.scalar.activation(tcn[:, cl], t_c[:, p, cl], AF.Tanh)
                      nc.vector.tensor_mul(hout[:, p, cl], tsig[:, 2, cl],
                                           tcn[:, cl])

                  for p in range(HK + 3):
                      if p < HK:
                          mms(p)
                          if p < HK - 2:
                              adds(p)
                          else:
                              # last two chunks: half-R ops so the
                              # step-boundary chain pipelines at finer grain
                              adds(p, HALVES[0])
                              adds(p, HALVES[1], alloc=False)
                      if 1 <= p and p - 1 < HK:
                          if p - 1 < HK - 2:
                              acts(p - 1)
                          else:
                              acts(p - 1, HALVES[0])
                              acts(p - 1, HALVES[1], alloc=False)
                      if p >= 3:
                          q = p - 3
                          if q < HK - 2:
                              tail(q)
                          else:
                              tail(q, HALVES[0])
                              tail(q, HALVES[1], alloc=False)
                  # output projection + sigmoid -> note (fp8 mirror first so
                  # the next step's Wn matmuls unblock as early as possible)
                  for tk in range(TK):
                      ts_ = slice(tk * 128, (tk + 1) * 128)
                      po = PSo.tile([128, R], F32, tag="dpo", name="dpo")
                      for k in range(0, HK, 2):
                          _mm_dr(nc, po[:], t_owt, hout, k, ts_,
                                 (k == 0), (k == HK - 2))
                      nc.scalar.activation(t_note[:, tk, :], po[:],
                                           AF.Sigmoid, bias=t_ob[:, tk:tk + 1])
                      if t + 1 < NS:
                          nc.vector.tensor_copy(t_note8[:, tk, :],
                                                t_note[:, tk, :])
                      nc.sync.dma_start(d["outbuf"][t, tk], t_note[:, tk, :])


import os
DEC_REPS = int(os.environ.get("KBENCH_DEC_REPS", "1"))
COND_REPS = int(os.environ.get("KBENCH_COND_REPS", "1"))

_CACHE = {}


def _build():
    if "nc" not in _CACHE:
        nc = bacc.Bacc("TRN2", target_bir_lowering=False, debug=False,
                       num_devices=NCORES)
        d = _declare(nc)
        with tile.TileContext(nc) as tc:
            _body(nc, tc, d)
        nc.compile()
        _CACHE["nc"] = nc
    return _CACHE["nc"]


def _feat_major(W, dt):
    """[J, K] -> [128, K/128, J] (stationary lhsT chunk layout)."""
    J, K = W.shape
    return np.ascontiguousarray(
        W.reshape(J, K // 128, 128).transpose(2, 1, 0)).astype(dt)


def _reorder4(W):
    """[4H, K] rows in PyTorch gate blocks (i,f,g,o) -> p-adjacent chunks in
    order (i,f,o,g): new chunk m = p*4 + {0:i,1:f,2:o,3:g}."""
    K = W.shape[1]
    W4 = W.reshape(4, HK, 128, K)[[0, 1, 3, 2]]
    return np.ascontiguousarray(W4.transpose(1, 0, 2, 3).reshape(4 * H, K))


def _pack_inputs(inputs):
    z = np.asarray(inputs["z"], np.float32)
    dec_h0 = np.asarray(inputs["dec_h0"], np.float32)
    dec_c0 = np.asarray(inputs["dec_c0"], np.float32)
    cond_b = np.asarray(inputs["cond_bih"] + inputs["cond_bhh"], np.float32)
    dec_b = np.asarray(inputs["dec_bih"] + inputs["dec_bhh"], np.float32)
    out_b = np.asarray(inputs["out_b"], np.float32)

    cb4 = cond_b.reshape(4, HK, 128)[[0, 1, 3, 2]]
    cbt = np.ascontiguousarray(cb4.transpose(1, 0, 2).reshape(G, 128).T)
    db4 = dec_b.reshape(4, HK, 128)
    gebb = np.ascontiguousarray(
        db4[[0, 1, 3]].transpose(1, 0, 2).reshape(1, 3 * H))
    dbg = np.ascontiguousarray(db4[2].T)

    We = np.asarray(inputs["dec_Wih"][:, :H], np.float32)
    We4 = We.reshape(4, HK, 128, H)
    We3 = np.ascontiguousarray(
        We4[[0, 1, 3]].transpose(1, 0, 2, 3).reshape(3 * H, H))
    dwe = _feat_major(We3, f8)                      # [128, HK, 3H]
    dweg = _feat_major(np.ascontiguousarray(We4[2].reshape(H, H)), f8)

    shared = {
        "cbt": cbt.astype(np.float32),
        "dbg": dbg.astype(np.float32),
        "ones": np.ones((1, Bc), dtype=bf16),
        "gebb": gebb.astype(bf16),
        "obias": np.ascontiguousarray(out_b.reshape(TK, 128).T).astype(np.float32),
        "cwih": _feat_major(_reorder4(np.asarray(inputs["cond_Wih"], np.float32)), f8),
        "cwhh": _feat_major(_reorder4(np.asarray(inputs["cond_Whh"], np.float32)), f8),
        "dwn": _feat_major(_reorder4(np.asarray(inputs["dec_Wih"][:, H:], np.float32)), f8),
        "dwhh": _feat_major(_reorder4(np.asarray(inputs["dec_Whh"], np.float32)), f8),
        "owt": _feat_major(np.asarray(inputs["out_W"], np.float32), f8),
        "dwe": dwe,
        "dweg": dweg,
    }

    z_lv = z[:, np.arange(L) * L, 0, :]           # [B, L, Z]
    in_maps = []
    for c in range(NCORES):
        bs = slice(c * Bc, (c + 1) * Bc)
        zc = z_lv[bs]                              # [Bc, L, Z]
        zT = np.ascontiguousarray(
            zc.reshape(Bc, L, ZK, 128).transpose(3, 2, 1, 0).reshape(128, ZK, R)
        ).astype(f8)
        h0 = dec_h0[:, bs, :]                      # [L, Bc, H]
        h0T = np.ascontiguousarray(
            h0.reshape(L, Bc, HK, 128).transpose(3, 2, 0, 1).reshape(128, HK, R))
        c0 = dec_c0[:, bs, :]
        c0T = np.ascontiguousarray(
            c0.reshape(L, Bc, HK, 128).transpose(3, 2, 0, 1).reshape(128, HK, R))
        m = dict(shared)
        m["zT"] = zT
        m["h0T"] = h0T.astype(f8)
        m["c0T"] = c0T.astype(np.float32)
        in_maps.append(m)
    return in_maps


def _unpack_outputs(core_outs):
    notes = np.empty((B, L * NS, T), np.float32)
    for c, arr in enumerate(core_outs):
        # arr [NS, TK, 128, R] -> [Bc, L, NS, T]
        a = arr.astype(np.float32).reshape(NS, TK, 128, L, Bc).transpose(4, 3, 0, 1, 2)
        notes[c * Bc:(c + 1) * Bc] = a.reshape(Bc, L, NS, T).reshape(
            Bc, L * NS, T)
    return notes


def kernel(**inputs):
    nc = _build()
    in_maps = _pack_inputs(inputs)
    res = run_bass_kernel_spmd(nc, in_maps, list(range(NCORES)))
    return _unpack_outputs([r["outbuf"] for r in res.results])
